# revision 13
# baseline (speedup 1.0000x reference)
"""DechirpSTFT Trainium2 kernel.

Math: the reference pipeline (hann window -> per-chirp lerp resample * jac
-> rfft(1024)) is linear in the windowed signal, so it folds into one
per-chirp matrix G_d[k, f2] (k = sample within window, f2 = interleaved
re/im of the 513 rfft bins).  The device kernel is the dense matmul

    out[f2, w] = sum_k G_d[k, f2] * x[b, 512*w + k]

computed G-stationary: each 128x128 tile of G is the PE weight operand and
all 512 (padded) windows stream through as the moving operand, contracting
k over 8 chunks of 128 into one PSUM bank per (b, f2-tile).

Dtype: bf16 operands (measured rel_rms 2.3e-3 vs the 2e-2 gate; fp8 e4m3
measured 2.7-3.7e-2 -> fails, so no DoubleRow double-pump).  bf16 streams
at the same 1 col/cycle as fp32r but halves DMA/SBUF traffic and allows
non-fp32 weight handling.  Output staged to bf16 (halves store traffic).

Schedule 'bf16ws' (weight-stationary): for each of the 16 weight tiles,
for each contraction chunk kc, the four batches stream b-inner so four
consecutive matmuls share one weight AP (walrus can elide reloads); each
(b, ft) PSUM bank accumulates one 8-chunk chain of N=512 columns.
Schedule 'base' is the previous fp32r h-outer variant kept for A/B.

Sharding: D=16 chirp rates, 2 per core across 8 cores (x replicated,
G sliced per core).  No cross-core communication.

Timing: no NTFF profiling is reachable through this axon client, so
test.py measures HW time as (wall(For_i(T)) - wall(For_i(1))) / (T-1)
with the jitted executable and device-resident inputs held across calls.
"""

import os
import sys

sys.path.insert(0, "/opt/trn_rl_repo")

import numpy as np
import ml_dtypes

# ---- problem constants (hardcoded; kernel.py must be self-contained) ----
B = 4
N = 262144
K = 1024
HOP = 512
NW = (N - K) // HOP + 1          # 511
KTAU = 1024
NF = 513                         # rfft bins
FW = 2 * NF                      # 1026 interleaved re/im
FE = 1024                        # live f2 cols per chirp (im0/imNyq dropped)
NWP = 512                        # padded window count
MQP = 513                        # padded m-quads so window 511 stays in-bounds
D = 16
NCORES = 8
D_PER = D // NCORES              # 2
KC = 8                           # contraction chunks of 128
MQ = N // 512                    # 512
NFT = 16                         # f2e weight tiles of 128 per core
EPS = 1e-8

SCHED = os.environ.get("KSCHED", "fft8")

_cache = {}


# --------------------------------------------------------------------------
# host-side G construction
# --------------------------------------------------------------------------
def _build_tables_np(dlnf):
    """Reference's per-chirp tables in numpy float32 (fallback path)."""
    dlnf = dlnf.astype(np.float32)
    beta = (2.0 * dlnf).astype(np.float32)
    small = np.abs(beta) < EPS
    beta_safe = np.where(small, np.float32(EPS), beta).astype(np.float32)
    e2b = np.exp(2.0 * beta_safe).astype(np.float32)

    tau = (2.0 * np.arange(KTAU, dtype=np.float32) / KTAU - 1.0).astype(np.float32)
    t_source = np.log(
        1.0 + (tau[None, :] + 1.0) / 2.0 * (e2b[:, None] - 1.0)
    ).astype(np.float32)
    t_source = (t_source / beta_safe[:, None] - 1.0).astype(np.float32)
    t_source = np.where(small[:, None], tau[None, :], t_source)

    tau_mid = np.float32(2.0 * (KTAU // 2) / KTAU - 1.0)
    t_mid = (
        np.log(1.0 + (tau_mid + 1.0) / 2.0 * (e2b - 1.0)) / beta_safe - 1.0
    ).astype(np.float32)
    t_mid = np.where(small, tau_mid, t_mid)

    jac = np.exp(-beta_safe[:, None] * (t_source - t_mid[:, None])).astype(np.float32)
    jac = np.where(small[:, None], np.float32(1.0), jac)

    idx = (np.float32(K / 2.0) * (t_source + 1.0)).astype(np.float32)
    idx_lo = np.clip(idx.astype(np.int32), 0, K - 2)
    frac = (idx - idx_lo.astype(np.float32)).astype(np.float32)
    return idx_lo, frac, jac


def _build_tables(dlnf):
    """Per-chirp tables, computed with jax on the CPU backend so the f32
    transcendentals (log/exp) match the reference bit-for-bit."""
    try:
        import jax
        import jax.numpy as jnp

        cpu = jax.devices("cpu")[0]
    except Exception:
        return _build_tables_np(dlnf)

    with jax.default_device(cpu):
        beta = 2.0 * jnp.asarray(dlnf, dtype=jnp.float32)
        small = jnp.abs(beta) < EPS
        beta_safe = jnp.where(small, EPS, beta)
        e2b = jnp.exp(2.0 * beta_safe)

        tau = 2.0 * jnp.arange(KTAU, dtype=jnp.float32) / KTAU - 1.0
        t_source = (
            jnp.log(1.0 + (tau[None, :] + 1.0) / 2.0 * (e2b[:, None] - 1.0))
            / beta_safe[:, None]
            - 1.0
        )
        t_source = jnp.where(small[:, None], tau[None, :], t_source)

        tau_mid = 2.0 * (KTAU // 2) / KTAU - 1.0
        t_mid = (
            jnp.log(1.0 + (tau_mid + 1.0) / 2.0 * (e2b - 1.0)) / beta_safe - 1.0
        )
        t_mid = jnp.where(small, tau_mid, t_mid)

        jac = jnp.exp(-beta_safe[:, None] * (t_source - t_mid[:, None]))
        jac = jnp.where(small[:, None], 1.0, jac)

        idx = (K / 2.0) * (t_source + 1.0)
        idx_lo = jnp.clip(idx.astype(jnp.int32), 0, K - 2)
        frac = idx - idx_lo.astype(jnp.float32)
    return np.asarray(idx_lo), np.asarray(frac), np.asarray(jac)


def _build_G(dlnf):
    """G[d, k, f2] f32: fused hann * lerp-resample * jac * rfft operator."""
    nd = dlnf.shape[0]
    idx_lo, frac, jac = _build_tables(dlnf)
    t = np.arange(KTAU, dtype=np.float64)
    f = np.arange(NF, dtype=np.float64)
    ang = 2.0 * np.pi * np.outer(t, f) / KTAU
    Wre = np.cos(ang)
    Wim = -np.sin(ang)
    n = np.arange(K, dtype=np.float32)
    hann = (0.5 * (1.0 - np.cos(2.0 * np.pi * n / K))).astype(np.float32)

    G = np.zeros((nd, K, FW), dtype=np.float64)
    for d in range(nd):
        c_lo = (jac[d] * (1.0 - frac[d])).astype(np.float64)
        c_hi = (jac[d] * frac[d]).astype(np.float64)
        Gre = np.zeros((K, NF))
        Gim = np.zeros((K, NF))
        np.add.at(Gre, idx_lo[d], c_lo[:, None] * Wre)
        np.add.at(Gim, idx_lo[d], c_lo[:, None] * Wim)
        np.add.at(Gre, idx_lo[d] + 1, c_hi[:, None] * Wre)
        np.add.at(Gim, idx_lo[d] + 1, c_hi[:, None] * Wim)
        G[d, :, 0::2] = Gre
        G[d, :, 1::2] = Gim
    G *= hann[None, :, None].astype(np.float64)
    return G.astype(np.float32)


# live G columns: drop im(bin0) (col 1) and im(Nyquist) (col 1025)
_KEEP = np.concatenate(([0], np.arange(2, 1025)))


# --------------------------------------------------------------------------
# fft8: factorized pipeline  r = L_d @ xw  ->  radix-8 combine (DVE)
#   -> per-residue folded DFT-128 (PE).  ~119K PE column-passes vs 262K
# for the dense-G matmul.  Math validated in mock_fft8.py (bf16 rel err
# 3.2e-3 vs the 2e-2 gate).
# --------------------------------------------------------------------------
NST = 8          # output stiles per slice: [s0, s4, s1t0, s1t1, s2t0.., s3t1]
NBW = 14         # stage-B weight tiles
SQ2H = float(np.sqrt(2.0) / 2.0)


def _build_L_dense(dlnf):
    """L[d, tau, n] f32 resample operator with hann folded."""
    idx_lo, frac, jac = _build_tables(dlnf)
    n = np.arange(K, dtype=np.float32)
    hann = (0.5 * (1.0 - np.cos(2.0 * np.pi * n / K))).astype(np.float32)
    nd = dlnf.shape[0]
    L = np.zeros((nd, K, K), dtype=np.float32)
    rows = np.arange(K)
    for d in range(nd):
        c_lo = jac[d] * (1.0 - frac[d])
        c_hi = jac[d] * frac[d]
        L[d, rows, idx_lo[d]] += c_lo * hann[idx_lo[d]]
        L[d, rows, idx_lo[d] + 1] += c_hi * hann[idx_lo[d] + 1]
    return L, idx_lo


def _fft8_bweights():
    """bw [128, 14, 128] f32: stage-B weights W_s'[t,q]=w1024^{ts} W128^{tq},
    columns packed (q, re/im); s=1 im-chunk sign-folded (operand is -u1im)."""
    if "bw" in _cache:
        return _cache["bw"]
    t = np.arange(128)[:, None]
    tiles = []
    for s in (0, 4):
        qs = np.arange(65) if s == 0 else np.arange(64)
        W = np.exp(-2j * np.pi * t * (8 * qs[None, :] + s) / 1024.0)
        cols = []
        for qi, q in enumerate(qs):
            if s == 0 and q in (0, 64):
                cols.append(W[:, qi].real)
            else:
                cols.append(W[:, qi].real)
                cols.append(W[:, qi].imag)
        tiles.append(np.stack(cols, 1))
    for s in (1, 2, 3):
        for ot in range(2):
            qs = np.arange(64) + 64 * ot
            W = np.exp(-2j * np.pi * t * (8 * qs[None, :] + s) / 1024.0)
            cre, cim = [], []
            for qi in range(64):
                cre.append(W[:, qi].real)
                cre.append(W[:, qi].imag)
                cim.append(-W[:, qi].imag)
                cim.append(W[:, qi].real)
            Wre = np.stack(cre, 1)
            Wim = np.stack(cim, 1)
            if s == 1:
                Wim = -Wim
            tiles.append(Wre)
            tiles.append(Wim)
    bw = np.stack(tiles, 1).astype(np.float32)  # [128, 14, 128]
    _cache["bw"] = bw
    return bw


# stage-B matmul list: (bw_idx, u_name, start, stop) per output stile
_BMMS = [
    [(0, "u0", True, True)],
    [(1, "u4", True, True)],
    [(2, "u1re", True, False), (3, "u1imn", False, True)],
    [(4, "u1re", True, False), (5, "u1imn", False, True)],
    [(6, "u2re", True, False), (7, "u2im", False, True)],
    [(8, "u2re", True, False), (9, "u2im", False, True)],
    [(10, "u3re", True, False), (11, "u3im", False, True)],
    [(12, "u3re", True, False), (13, "u3im", False, True)],
]
_ST_S = [(0, 0), (4, 0), (1, 0), (1, 1), (2, 0), (2, 1), (3, 0), (3, 1)]


def _fft8_maps():
    """Static per-stile unpack maps: row -> (k, comp, sign)."""
    if "maps" in _cache:
        return _cache["maps"]
    maps = []
    for st in range(NST):
        s, ot = _ST_S[st]
        kk = np.zeros(128, np.int64)
        cc = np.zeros(128, np.int64)
        sg = np.zeros(128, np.float32)
        if s == 0:
            row = 0
            for q in range(65):
                if q in (0, 64):
                    kk[row], cc[row], sg[row] = 8 * q, 0, 1.0
                    row += 1
                else:
                    kk[row], cc[row], sg[row] = 8 * q, 0, 1.0
                    kk[row + 1], cc[row + 1], sg[row + 1] = 8 * q, 1, 1.0
                    row += 2
            assert row == 128
        elif s == 4:
            for qi in range(64):
                kk[2 * qi], cc[2 * qi], sg[2 * qi] = 8 * qi + 4, 0, 1.0
                kk[2 * qi + 1], cc[2 * qi + 1], sg[2 * qi + 1] = 8 * qi + 4, 1, 1.0
        else:
            for qi in range(64):
                q = qi + 64 * ot
                k = 8 * q + s
                if k <= 512:
                    kk[2 * qi], cc[2 * qi], sg[2 * qi] = k, 0, 1.0
                    kk[2 * qi + 1], cc[2 * qi + 1], sg[2 * qi + 1] = k, 1, 1.0
                else:
                    km = 1024 - k
                    assert 0 <= km <= 512
                    kk[2 * qi], cc[2 * qi], sg[2 * qi] = km, 0, 1.0
                    kk[2 * qi + 1], cc[2 * qi + 1], sg[2 * qi + 1] = km, 1, -1.0
        maps.append((kk, cc, sg))
    _cache["maps"] = maps
    return maps


def _fft8_structure(dlnf):
    """Per-core chain structure + L-weight blocks (lhsT [n,tau] layout)."""
    L, idx_lo = _build_L_dense(dlnf)
    cores = []
    for c in range(NCORES):
        chains = []  # (dl, I, [J...], blk_base)
        blocks = []
        for dl in range(D_PER):
            d = D_PER * c + dl
            for I in range(8):
                seg = idx_lo[d, 128 * I : 128 * (I + 1)]
                lo = int(seg.min()) // 128
                hi = int(seg.max() + 1) // 128
                js = list(range(lo, hi + 1))
                chains.append((dl, I, js, len(blocks)))
                for J in js:
                    blk = L[d, 128 * I : 128 * (I + 1), 128 * J : 128 * (J + 1)]
                    blocks.append(np.ascontiguousarray(blk.T))  # [n, tau]
        lw = np.stack(blocks, 1).astype(ml_dtypes.bfloat16)  # [128, nblk, 128]
        cores.append({"chains": chains, "lw": lw, "nblk": len(blocks)})
    return cores


def _build_nc_fft8(struct, iters=1):
    import concourse.bacc as bacc
    import concourse.mybir as mybir
    from concourse import tile

    f32 = mybir.dt.float32
    bf16 = mybir.dt.bfloat16
    nblk = struct["nblk"]
    chains = struct["chains"]

    nc = bacc.Bacc("TRN2", target_bir_lowering=False, debug=False)
    xt_d = nc.dram_tensor("xt", [128, B, 4, MQP], bf16, kind="ExternalInput")
    lw_d = nc.dram_tensor("lw", [128, nblk, 128], bf16, kind="ExternalInput")
    bw_d = nc.dram_tensor("bw", [128, NBW, 128], bf16, kind="ExternalInput")
    # out[slice = 4*dl + b, stile, p, w]
    out_d = nc.dram_tensor("out", [8, NST, 128, NWP], bf16, kind="ExternalOutput")

    A = mybir.AluOpType

    def body(nc, tc, pools):
        (xpool, lwpool, rsb, upool, tpool, xst, rps, bps) = pools
        x_sb = xpool.tile([128, B, 4, MQP], bf16, name="x_sb")
        lw_sb = lwpool.tile([128, nblk, 128], bf16, name="lw_sb")
        bw_sb = lwpool.tile([128, NBW, 128], bf16, name="bw_sb")
        nc.sync.dma_start(lw_sb, lw_d[:])
        for b in range(B):
            nc.sync.dma_start(x_sb[:, b], xt_d[:, b])
        nc.sync.dma_start(bw_sb, bw_d[:])

        rtiles = [None, None]  # per pipeline parity: dict I -> sbuf tile
        utiles = [None, None]

        def emit_R(s):
            dl, b = s // 4, s % 4
            rt = {}
            for (cdl, I, js, base) in chains:
                if cdl != dl:
                    continue
                ps = rps.tile([128, NWP], f32, name="rp", tag="rp")
                for ci, J in enumerate(js):
                    nc.tensor.matmul(
                        ps,
                        lw_sb[:, base + ci],
                        x_sb[:, b, J % 4, J // 4 : J // 4 + NWP],
                        start=(ci == 0),
                        stop=(ci == len(js) - 1),
                    )
                rsbt = rsb.tile([128, NWP], f32, name=f"r{I}", tag=f"r{I}")
                nc.scalar.copy(rsbt, ps)
                rt[I] = rsbt
            rtiles[s % 2] = rt

        def emit_A(s):
            r = rtiles[s % 2]
            t = {
                nm: tpool.tile([128, NWP], f32, name=nm, tag=nm)
                for nm in (
                    "e0", "e1", "e2", "e3", "o0", "o1", "o2", "o3",
                    "t1", "t2", "d1", "d2",
                )
            }
            u = {
                nm: upool.tile([128, NWP], bf16, name=nm, tag=nm)
                for nm in (
                    "u0", "u4", "u1re", "u1imn", "u2re", "u2im", "u3re", "u3im"
                )
            }
            v = nc.vector
            for j in range(4):
                v.tensor_tensor(t[f"e{j}"], r[j], r[j + 4], op=A.add)
            for j in range(4):
                v.tensor_tensor(t[f"o{j}"], r[j], r[j + 4], op=A.subtract)
            v.tensor_tensor(t["t1"], t["e0"], t["e2"], op=A.add)
            v.tensor_tensor(t["t2"], t["e1"], t["e3"], op=A.add)
            v.tensor_tensor(u["u0"], t["t1"], t["t2"], op=A.add)
            v.tensor_tensor(u["u4"], t["t1"], t["t2"], op=A.subtract)
            v.tensor_tensor(u["u2re"], t["e0"], t["e2"], op=A.subtract)
            v.tensor_tensor(u["u2im"], t["e3"], t["e1"], op=A.subtract)
            v.tensor_tensor(t["d1"], t["o1"], t["o3"], op=A.subtract)
            v.tensor_tensor(t["d2"], t["o1"], t["o3"], op=A.add)
            # u1re = k*d1 + o0 ; u3re = -k*d1 + o0
            v.scalar_tensor_tensor(u["u1re"], t["d1"], SQ2H, t["o0"], A.mult, A.add)
            v.scalar_tensor_tensor(u["u3re"], t["d1"], -SQ2H, t["o0"], A.mult, A.add)
            # u1imn = k*d2 + o2 (= -u1im, sign folded in bw); u3im = -k*d2 + o2
            v.scalar_tensor_tensor(u["u1imn"], t["d2"], SQ2H, t["o2"], A.mult, A.add)
            v.scalar_tensor_tensor(u["u3im"], t["d2"], -SQ2H, t["o2"], A.mult, A.add)
            utiles[s % 2] = u

        def emit_B(s):
            u = utiles[s % 2]
            for st in range(NST):
                ps = bps.tile([128, NWP], f32, name="xp", tag="xp")
                for (bwi, unm, sa, so) in _BMMS[st]:
                    nc.tensor.matmul(
                        ps, bw_sb[:, bwi], u[unm], start=sa, stop=so
                    )
                xs = xst.tile([128, NWP], bf16, name="xs", tag="xs")
                nc.vector.tensor_copy(xs, ps)
                nc.scalar.dma_start(out_d[s, st], xs)

        emit_R(0)
        emit_A(0)
        emit_R(1)
        for s in range(8):
            emit_B(s)
            if s + 1 < 8:
                emit_A(s + 1)
            if s + 2 < 8:
                emit_R(s + 2)

    with tile.TileContext(nc) as tc:
        with (
            tc.tile_pool(name="xsb", bufs=2) as xpool,
            tc.tile_pool(name="lwsb", bufs=2) as lwpool,
            tc.tile_pool(name="rsb", bufs=2) as rsb,
            tc.tile_pool(name="usb", bufs=2) as upool,
            tc.tile_pool(name="tsb", bufs=1) as tpool,
            tc.tile_pool(name="xst", bufs=6) as xst,
            tc.tile_pool(name="rps", bufs=4, space="PSUM") as rps,
            tc.tile_pool(name="bps", bufs=4, space="PSUM") as bps,
        ):
            pools = (xpool, lwpool, rsb, upool, tpool, xst, rps, bps)
            if iters > 1:
                with tc.For_i(0, iters, 1):
                    body(nc, tc, pools)
            else:
                body(nc, tc, pools)

    nc.compile()
    return nc


def _get_fft8_ncs(dlnf, iters):
    key = ("fft8", dlnf.tobytes(), iters)
    if key not in _cache:
        if ("fft8s", dlnf.tobytes()) not in _cache:
            _cache[("fft8s", dlnf.tobytes())] = _fft8_structure(dlnf)
        structs = _cache[("fft8s", dlnf.tobytes())]
        _cache[key] = [_build_nc_fft8(s, iters) for s in structs]
    return _cache[key]


def _prep_fft8(x, dlnf):
    x = np.asarray(x, dtype=np.float32)
    dlnf = np.asarray(dlnf, dtype=np.float32)
    skey = ("fft8s", dlnf.tobytes())
    if skey not in _cache:
        _cache[skey] = _fft8_structure(dlnf)
    structs = _cache[skey]
    xt_n = x.reshape(B, MQ, 4, 128).transpose(3, 0, 2, 1)
    xt = np.zeros((128, B, 4, MQP), ml_dtypes.bfloat16)
    xt[:, :, :, :MQ] = xt_n.astype(ml_dtypes.bfloat16)
    xt = np.ascontiguousarray(xt)
    bw = _fft8_bweights().astype(ml_dtypes.bfloat16)
    return [
        {"xt": xt, "lw": np.ascontiguousarray(structs[c]["lw"]), "bw": bw}
        for c in range(NCORES)
    ]


def _assemble_fft8(results):
    maps = _fft8_maps()
    full = np.zeros((B, NW, D, NF, 2), dtype=np.float32)
    for c, r in enumerate(results):
        o = np.asarray(r["out"]).astype(np.float32)  # [8, 8, 128, NWP]
        for dl in range(D_PER):
            d = D_PER * c + dl
            for b in range(B):
                sl = o[4 * dl + b]  # [8, 128, NWP]
                for st in range(NST):
                    kk, cc, sg = maps[st]
                    full[b, :, d, kk, cc] = (
                        sl[st, :, :NW] * sg[:, None]
                    )
    return (
        full.reshape(B, NW, D, NF * 2)
        .view(np.complex64)
        .reshape(B, NW, D, NF)
    )


# --------------------------------------------------------------------------
# device program
# --------------------------------------------------------------------------
def _build_nc(iters=1, sched=None):
    import concourse.bacc as bacc
    import concourse.mybir as mybir
    from concourse import tile

    sched = sched or SCHED
    f32 = mybir.dt.float32
    bf16 = mybir.dt.bfloat16
    mm_dt = mybir.dt.float32r if sched == "base" else bf16

    nc = bacc.Bacc("TRN2", target_bir_lowering=False, debug=False)

    # xt[p, b, r, mq] = x[b, 128*(4*mq + r) + p]  (mq innermost: every
    # matmul moving slice is contiguous)
    xt_d = nc.dram_tensor("xt", [128, B, 4, MQP], mm_dt, kind="ExternalInput")
    # g[p, kc, 1024*d + fe] = G_d[128*kc + p, keep[fe]]
    g_d = nc.dram_tensor("g", [128, KC, D_PER * FE], mm_dt, kind="ExternalInput")
    if sched == "base":
        out_d = nc.dram_tensor(
            "out", [B, NFT // 4, 128, 4, NWP], f32, kind="ExternalOutput"
        )
    else:
        # out[b, fg, p, j, w]: f2e tile ft = 2*fg + j, psum partition p
        out_d = nc.dram_tensor(
            "out", [B, NFT // 2, 128, 2, NWP], bf16, kind="ExternalOutput"
        )

    def body_base(nc, tc, xpool, gpool, spool, ppool):
        x_sb = xpool.tile([128, B, 4, MQP], mm_dt, name="x_sb")
        g_sb = gpool.tile([128, KC, D_PER * FE], mm_dt, name="g_sb")
        for b in range(B):
            nc.sync.dma_start(x_sb[:, b], xt_d[:, b])
        for kc in range(KC):
            nc.sync.dma_start(g_sb[:, kc], g_d[:, kc])

        for b in range(B):
            for ftp in range(NFT // 2):
                st = spool.tile([128, 2, NWP], f32, name="st")
                for jj in range(2):
                    ft = 2 * ftp + jj
                    ps = ppool.tile([128, NWP], f32, name="ps", tag="ps")
                    for h in range(2):
                        for kc in range(KC):
                            q, r = divmod(kc, 4)
                            nc.tensor.matmul(
                                ps[:, h * 256 : (h + 1) * 256],
                                g_sb[:, kc, 128 * ft : 128 * (ft + 1)],
                                x_sb[:, b, r, q + h * 256 : q + h * 256 + 256],
                                start=(kc == 0 and h == 0),
                                stop=(kc == KC - 1 and h == 1),
                            )
                    eng = nc.vector.tensor_copy if ft % 2 == 0 else nc.scalar.copy
                    eng(st[:, jj], ps)
                nc.scalar.dma_start(
                    out_d[b, ftp // 2, :, 2 * (ftp % 2) : 2 * (ftp % 2) + 2], st
                )

    def body_ws(nc, tc, xpool, gpool, spool, ppool):
        x_sb = xpool.tile([128, B, 4, MQP], mm_dt, name="x_sb")
        g_sb = gpool.tile([128, KC, D_PER * FE], mm_dt, name="g_sb")
        nc.sync.dma_start(g_sb[:, 0], g_d[:, 0])
        for b in range(B):
            nc.sync.dma_start(x_sb[:, b], xt_d[:, b])
        for kc in range(1, KC):
            nc.sync.dma_start(g_sb[:, kc], g_d[:, kc])

        st = {}
        for ftp in range(NFT):
            ps = [
                ppool.tile([128, NWP], f32, name=f"ps{b}", tag=f"ps{b}")
                for b in range(B)
            ]
            for kc in range(KC):
                q, r = divmod(kc, 4)
                w_ap = g_sb[:, kc, 128 * ftp : 128 * (ftp + 1)]
                for b in range(B):
                    nc.tensor.matmul(
                        ps[b],
                        w_ap,
                        x_sb[:, b, r, q : q + NWP],
                        start=(kc == 0),
                        stop=(kc == KC - 1),
                    )
            jj = ftp % 2
            if jj == 0:
                for b in range(B):
                    st[b] = spool.tile([128, 2, NWP], bf16, name=f"st{b}")
            for b in range(B):
                eng = nc.vector.tensor_copy if b % 2 == 0 else nc.scalar.copy
                eng(st[b][:, jj], ps[b])
            if jj == 1:
                for b in range(B):
                    nc.scalar.dma_start(out_d[b, ftp // 2], st[b])

    body = body_base if sched == "base" else body_ws

    with tile.TileContext(nc) as tc:
        with (
            tc.tile_pool(name="xsb", bufs=2) as xpool,
            tc.tile_pool(name="gsb", bufs=2) as gpool,
            tc.tile_pool(name="stage", bufs=3) as spool,
            tc.tile_pool(
                name="psum", bufs=8 if sched == "base" else 2, space="PSUM"
            ) as ppool,
        ):
            if iters > 1:
                with tc.For_i(0, iters, 1):
                    body(nc, tc, xpool, gpool, spool, ppool)
            else:
                body(nc, tc, xpool, gpool, spool, ppool)

    nc.compile()
    return nc


def _get_nc(iters=1, sched=None):
    sched = sched or SCHED
    key = ("nc", iters, sched)
    if key not in _cache:
        _cache[key] = _build_nc(iters, sched)
    return _cache[key]


# --------------------------------------------------------------------------
# host prep / assembly
# --------------------------------------------------------------------------
def _prep_arrays(x, dlnf, sched=None):
    """Host prep: G matrices + transposed/sharded device input arrays."""
    sched = sched or SCHED
    dt = np.float32 if sched == "base" else ml_dtypes.bfloat16
    x = np.asarray(x, dtype=np.float32)
    dlnf = np.asarray(dlnf, dtype=np.float32)
    G = _build_G(dlnf)                                     # (16, 1024, 1026)
    xt_n = x.reshape(B, MQ, 4, 128).transpose(3, 0, 2, 1)  # (128, B, 4, MQ)
    xt = np.zeros((128, B, 4, MQP), dt)
    xt[:, :, :, :MQ] = xt_n.astype(dt)
    xt = np.ascontiguousarray(xt)
    Ge = G[:, :, _KEEP]                                    # (16, 1024, 1024)
    g_all = Ge.reshape(D, KC, 128, FE).transpose(2, 1, 0, 3)  # (128,KC,D,FE)
    in_maps = [
        {
            "xt": xt,
            "g": np.ascontiguousarray(
                g_all[:, :, c * D_PER : (c + 1) * D_PER]
                .reshape(128, KC, D_PER * FE)
                .astype(dt)
            ),
        }
        for c in range(NCORES)
    ]
    return in_maps


def _assemble(results, sched=None):
    """per-core out2 -> (B, NW, D, NF) complex64."""
    sched = sched or SCHED
    full = np.zeros((B, NW, D, FW), dtype=np.float32)
    for c, r in enumerate(results):
        o = np.asarray(r["out"]).astype(np.float32)[..., :NW]
        o = o.transpose(0, 4, 1, 3, 2).reshape(B, NW, D_PER, FE)
        for dd in range(D_PER):
            full[:, :, c * D_PER + dd, _KEEP] = o[:, :, dd]
    return full.view(np.complex64).reshape(B, NW, D, NF)


# --------------------------------------------------------------------------
# runner (jitted multi-core executable, cached across kernel() calls)
# --------------------------------------------------------------------------
def _make_sharded(nc, devices=None):
    import jax
    from jax.experimental.shard_map import shard_map
    from jax.sharding import Mesh, PartitionSpec

    from concourse import bass2jax as b2j
    import concourse.mybir as mybir

    b2j.install_neuronx_cc_hook()
    partition_name = nc.partition_id_tensor.name if nc.partition_id_tensor else None

    in_names, out_names, out_avals, zero_outs = [], [], [], []
    for alloc in nc.m.functions[0].allocations:
        if not isinstance(alloc, mybir.MemoryLocationSet):
            continue
        name = alloc.memorylocations[0].name
        if alloc.kind == "ExternalInput":
            if name != partition_name:
                in_names.append(name)
        elif alloc.kind == "ExternalOutput":
            out_names.append(name)
            shape = tuple(alloc.tensor_shape)
            dtype = mybir.dt.np(alloc.dtype)
            out_avals.append(jax.core.ShapedArray(shape, dtype))
            zero_outs.append(np.zeros(shape, dtype))
    all_names = in_names + out_names
    if partition_name is not None:
        all_names = all_names + [partition_name]

    def _body(*args):
        operands = list(args)
        if partition_name is not None:
            operands.append(b2j.partition_id_tensor())
        outs = b2j._bass_exec_p.bind(
            *operands,
            out_avals=tuple(out_avals),
            in_names=tuple(all_names),
            out_names=tuple(out_names),
            lowering_input_output_aliases=(),
            sim_require_finite=True,
            sim_require_nnan=True,
            nc=nc,
        )
        return tuple(outs)

    if devices is None:
        devices = jax.devices()[:NCORES]
    mesh = Mesh(np.asarray(devices), ("core",))
    nin = len(in_names) + len(zero_outs)
    sharded = jax.jit(
        shard_map(
            _body,
            mesh=mesh,
            in_specs=(PartitionSpec("core"),) * nin,
            out_specs=(PartitionSpec("core"),) * len(out_names),
            check_rep=False,
        ),
        keep_unused=True,
    )
    return sharded, in_names, out_names, out_avals, zero_outs


def _get_runner(iters, sched=None):
    sched = sched or SCHED
    key = ("runner", iters, sched)
    if key in _cache:
        return _cache[key]

    import jax

    nc = _get_nc(iters, sched)
    sharded, in_names, out_names, out_avals, zero_outs = _make_sharded(nc)

    def call(in_maps):
        concat_in = [
            np.concatenate([in_maps[c][name] for c in range(NCORES)], axis=0)
            for name in in_names
        ] + [
            np.zeros((NCORES * z.shape[0], *z.shape[1:]), z.dtype)
            for z in zero_outs
        ]
        out_arrs = sharded(*concat_in)
        jax.block_until_ready(out_arrs)
        return [
            {
                name: np.asarray(out_arrs[i]).reshape(
                    NCORES, *out_avals[i].shape
                )[c]
                for i, name in enumerate(out_names)
            }
            for c in range(NCORES)
        ]

    _cache[key] = call
    return call


def _get_fft8_runner(dlnf, iters):
    """Heterogeneous per-core programs: 8 single-device executables."""
    key = ("fft8run", dlnf.tobytes(), iters)
    if key in _cache:
        return _cache[key]

    import jax

    ncs = _get_fft8_ncs(dlnf, iters)
    devices = jax.devices()[:NCORES]
    cores = []
    for c in range(NCORES):
        sharded, in_names, out_names, out_avals, zero_outs = _make_sharded(
            ncs[c], devices=[devices[c]]
        )
        cores.append((sharded, in_names, out_names, out_avals, zero_outs))

    def call(in_maps):
        outs = []
        for c in range(NCORES):
            sharded, in_names, out_names, out_avals, zero_outs = cores[c]
            args = [
                jax.device_put(in_maps[c][n], devices[c]) for n in in_names
            ] + [
                jax.device_put(np.zeros(z.shape, z.dtype), devices[c])
                for z in zero_outs
            ]
            outs.append(sharded(*args))
        jax.block_until_ready(outs)
        results = []
        for c in range(NCORES):
            _, _, out_names, out_avals, _ = cores[c]
            results.append(
                {
                    n: np.asarray(outs[c][i]).reshape(out_avals[i].shape)
                    for i, n in enumerate(out_names)
                }
            )
        return results

    _cache[key] = call
    return call


def kernel(x, dlnf, n_hann_splits=1, **_unused):
    iters = int(os.environ.get("KERNEL_ITERS", "1"))
    dlnf32 = np.asarray(dlnf, dtype=np.float32)
    if SCHED == "fft8":
        try:
            in_maps = _prep_fft8(x, dlnf32)
            call = _get_fft8_runner(dlnf32, iters)
            return _assemble_fft8(call(in_maps))
        except Exception:
            import traceback

            traceback.print_exc()
            # fall through to the dense-G path
    in_maps = _prep_arrays(x, dlnf, sched="bf16ws")
    try:
        call = _get_runner(iters, sched="bf16ws")
        results = call(in_maps)
    except Exception:
        # robust fallback: the reference implementation of the SPMD runner
        from concourse.bass_utils import run_bass_kernel_spmd

        nc = _get_nc(iters, sched="bf16ws")
        res = run_bass_kernel_spmd(nc, in_maps, core_ids=list(range(NCORES)))
        results = res.results
    return _assemble(results, sched="bf16ws")


# --------------------------------------------------------------------------
# benchmarking: jit once, time repeated executions (no retrace/relower)
# --------------------------------------------------------------------------
def prepare_bench(x, dlnf, iters, sched=None):
    """Returns run() -> wall seconds for one execution of the iters-body NEFF."""
    import time

    import jax

    sched = sched or SCHED
    if sched == "fft8":
        dlnf32 = np.asarray(dlnf, dtype=np.float32)
        in_maps = _prep_fft8(x, dlnf32)
        ncs = _get_fft8_ncs(dlnf32, iters)
        devices = jax.devices()[:NCORES]
        cores = []
        for c in range(NCORES):
            sharded, in_names, out_names, out_avals, zero_outs = _make_sharded(
                ncs[c], devices=[devices[c]]
            )
            args = [
                jax.device_put(in_maps[c][n], devices[c]) for n in in_names
            ] + [
                jax.device_put(np.zeros(z.shape, z.dtype), devices[c])
                for z in zero_outs
            ]
            cores.append((sharded, args))
        outs = [s(*a) for s, a in cores]
        jax.block_until_ready(outs)

        def run():
            t0 = time.perf_counter()
            o = [s(*a) for s, a in cores]
            jax.block_until_ready(o)
            return time.perf_counter() - t0

        return run
    in_maps = _prep_arrays(x, dlnf, sched)
    nc = _get_nc(iters, sched)
    sharded, in_names, out_names, out_avals, zero_outs = _make_sharded(nc)
    concat_in = [
        np.concatenate([in_maps[c][name] for c in range(NCORES)], axis=0)
        for name in in_names
    ] + [np.zeros((NCORES * z.shape[0], *z.shape[1:]), z.dtype) for z in zero_outs]
    concat_in = [jax.device_put(a) for a in concat_in]

    out = sharded(*concat_in)
    jax.block_until_ready(out)

    def run():
        t0 = time.perf_counter()
        o = sharded(*concat_in)
        jax.block_until_ready(o)
        return time.perf_counter() - t0

    return run


if __name__ == "__main__":
    rng = np.random.default_rng(0)
    x = rng.standard_normal((B, N), dtype=np.float32)
    dlnf = rng.uniform(-0.5, 0.5, size=(D,)).astype(np.float32)
    out = kernel(x, dlnf, 1)
    print("out:", out.shape, out.dtype)


# revision 15
# speedup vs baseline: 1.1983x; 1.1983x over previous
"""DechirpSTFT Trainium2 kernel.

Math: the reference pipeline (hann window -> per-chirp lerp resample * jac
-> rfft(1024)) is linear in the windowed signal, so it folds into one
per-chirp matrix G_d[k, f2] (k = sample within window, f2 = interleaved
re/im of the 513 rfft bins).  The device kernel is the dense matmul

    out[f2, w] = sum_k G_d[k, f2] * x[b, 512*w + k]

computed G-stationary: each 128x128 tile of G is the PE weight operand and
all 512 (padded) windows stream through as the moving operand, contracting
k over 8 chunks of 128 into one PSUM bank per (b, f2-tile).

Dtype: bf16 operands (measured rel_rms 2.3e-3 vs the 2e-2 gate; fp8 e4m3
measured 2.7-3.7e-2 -> fails, so no DoubleRow double-pump).  bf16 streams
at the same 1 col/cycle as fp32r but halves DMA/SBUF traffic and allows
non-fp32 weight handling.  Output staged to bf16 (halves store traffic).

Schedule 'bf16ws' (weight-stationary): for each of the 16 weight tiles,
for each contraction chunk kc, the four batches stream b-inner so four
consecutive matmuls share one weight AP (walrus can elide reloads); each
(b, ft) PSUM bank accumulates one 8-chunk chain of N=512 columns.
Schedule 'base' is the previous fp32r h-outer variant kept for A/B.

Sharding: D=16 chirp rates, 2 per core across 8 cores (x replicated,
G sliced per core).  No cross-core communication.

Timing: no NTFF profiling is reachable through this axon client, so
test.py measures HW time as (wall(For_i(T)) - wall(For_i(1))) / (T-1)
with the jitted executable and device-resident inputs held across calls.
"""

import os
import sys

sys.path.insert(0, "/opt/trn_rl_repo")

import numpy as np
import ml_dtypes

# ---- problem constants (hardcoded; kernel.py must be self-contained) ----
B = 4
N = 262144
K = 1024
HOP = 512
NW = (N - K) // HOP + 1          # 511
KTAU = 1024
NF = 513                         # rfft bins
FW = 2 * NF                      # 1026 interleaved re/im
FE = 1024                        # live f2 cols per chirp (im0/imNyq dropped)
NWP = 512                        # padded window count
MQP = 513                        # padded m-quads so window 511 stays in-bounds
D = 16
NCORES = 8
D_PER = D // NCORES              # 2
KC = 8                           # contraction chunks of 128
MQ = N // 512                    # 512
NFT = 16                         # f2e weight tiles of 128 per core
EPS = 1e-8

SCHED = os.environ.get("KSCHED", "fft8")

_cache = {}


# --------------------------------------------------------------------------
# host-side G construction
# --------------------------------------------------------------------------
def _build_tables_np(dlnf):
    """Reference's per-chirp tables in numpy float32 (fallback path)."""
    dlnf = dlnf.astype(np.float32)
    beta = (2.0 * dlnf).astype(np.float32)
    small = np.abs(beta) < EPS
    beta_safe = np.where(small, np.float32(EPS), beta).astype(np.float32)
    e2b = np.exp(2.0 * beta_safe).astype(np.float32)

    tau = (2.0 * np.arange(KTAU, dtype=np.float32) / KTAU - 1.0).astype(np.float32)
    t_source = np.log(
        1.0 + (tau[None, :] + 1.0) / 2.0 * (e2b[:, None] - 1.0)
    ).astype(np.float32)
    t_source = (t_source / beta_safe[:, None] - 1.0).astype(np.float32)
    t_source = np.where(small[:, None], tau[None, :], t_source)

    tau_mid = np.float32(2.0 * (KTAU // 2) / KTAU - 1.0)
    t_mid = (
        np.log(1.0 + (tau_mid + 1.0) / 2.0 * (e2b - 1.0)) / beta_safe - 1.0
    ).astype(np.float32)
    t_mid = np.where(small, tau_mid, t_mid)

    jac = np.exp(-beta_safe[:, None] * (t_source - t_mid[:, None])).astype(np.float32)
    jac = np.where(small[:, None], np.float32(1.0), jac)

    idx = (np.float32(K / 2.0) * (t_source + 1.0)).astype(np.float32)
    idx_lo = np.clip(idx.astype(np.int32), 0, K - 2)
    frac = (idx - idx_lo.astype(np.float32)).astype(np.float32)
    return idx_lo, frac, jac


def _build_tables(dlnf):
    """Per-chirp tables, computed with jax on the CPU backend so the f32
    transcendentals (log/exp) match the reference bit-for-bit."""
    try:
        import jax
        import jax.numpy as jnp

        cpu = jax.devices("cpu")[0]
    except Exception:
        return _build_tables_np(dlnf)

    with jax.default_device(cpu):
        beta = 2.0 * jnp.asarray(dlnf, dtype=jnp.float32)
        small = jnp.abs(beta) < EPS
        beta_safe = jnp.where(small, EPS, beta)
        e2b = jnp.exp(2.0 * beta_safe)

        tau = 2.0 * jnp.arange(KTAU, dtype=jnp.float32) / KTAU - 1.0
        t_source = (
            jnp.log(1.0 + (tau[None, :] + 1.0) / 2.0 * (e2b[:, None] - 1.0))
            / beta_safe[:, None]
            - 1.0
        )
        t_source = jnp.where(small[:, None], tau[None, :], t_source)

        tau_mid = 2.0 * (KTAU // 2) / KTAU - 1.0
        t_mid = (
            jnp.log(1.0 + (tau_mid + 1.0) / 2.0 * (e2b - 1.0)) / beta_safe - 1.0
        )
        t_mid = jnp.where(small, tau_mid, t_mid)

        jac = jnp.exp(-beta_safe[:, None] * (t_source - t_mid[:, None]))
        jac = jnp.where(small[:, None], 1.0, jac)

        idx = (K / 2.0) * (t_source + 1.0)
        idx_lo = jnp.clip(idx.astype(jnp.int32), 0, K - 2)
        frac = idx - idx_lo.astype(jnp.float32)
    return np.asarray(idx_lo), np.asarray(frac), np.asarray(jac)


def _build_G(dlnf):
    """G[d, k, f2] f32: fused hann * lerp-resample * jac * rfft operator."""
    nd = dlnf.shape[0]
    idx_lo, frac, jac = _build_tables(dlnf)
    t = np.arange(KTAU, dtype=np.float64)
    f = np.arange(NF, dtype=np.float64)
    ang = 2.0 * np.pi * np.outer(t, f) / KTAU
    Wre = np.cos(ang)
    Wim = -np.sin(ang)
    n = np.arange(K, dtype=np.float32)
    hann = (0.5 * (1.0 - np.cos(2.0 * np.pi * n / K))).astype(np.float32)

    G = np.zeros((nd, K, FW), dtype=np.float64)
    for d in range(nd):
        c_lo = (jac[d] * (1.0 - frac[d])).astype(np.float64)
        c_hi = (jac[d] * frac[d]).astype(np.float64)
        Gre = np.zeros((K, NF))
        Gim = np.zeros((K, NF))
        np.add.at(Gre, idx_lo[d], c_lo[:, None] * Wre)
        np.add.at(Gim, idx_lo[d], c_lo[:, None] * Wim)
        np.add.at(Gre, idx_lo[d] + 1, c_hi[:, None] * Wre)
        np.add.at(Gim, idx_lo[d] + 1, c_hi[:, None] * Wim)
        G[d, :, 0::2] = Gre
        G[d, :, 1::2] = Gim
    G *= hann[None, :, None].astype(np.float64)
    return G.astype(np.float32)


# live G columns: drop im(bin0) (col 1) and im(Nyquist) (col 1025)
_KEEP = np.concatenate(([0], np.arange(2, 1025)))


# --------------------------------------------------------------------------
# fft8: factorized pipeline  r = L_d @ xw  ->  radix-8 combine (DVE)
#   -> per-residue folded DFT-128 (PE).  ~119K PE column-passes vs 262K
# for the dense-G matmul.  Math validated in mock_fft8.py (bf16 rel err
# 3.2e-3 vs the 2e-2 gate).
# --------------------------------------------------------------------------
NST = 8          # output stiles per slice: [s0, s4, s1t0, s1t1, s2t0.., s3t1]
NBW = 14         # stage-B weight tiles
SQ2H = float(np.sqrt(2.0) / 2.0)


def _build_L_dense(dlnf):
    """L[d, tau, n] f32 resample operator with hann folded."""
    idx_lo, frac, jac = _build_tables(dlnf)
    n = np.arange(K, dtype=np.float32)
    hann = (0.5 * (1.0 - np.cos(2.0 * np.pi * n / K))).astype(np.float32)
    nd = dlnf.shape[0]
    L = np.zeros((nd, K, K), dtype=np.float32)
    rows = np.arange(K)
    for d in range(nd):
        c_lo = jac[d] * (1.0 - frac[d])
        c_hi = jac[d] * frac[d]
        L[d, rows, idx_lo[d]] += c_lo * hann[idx_lo[d]]
        L[d, rows, idx_lo[d] + 1] += c_hi * hann[idx_lo[d] + 1]
    return L, idx_lo


def _fft8_bweights():
    """bw [128, 14, 128] f32: stage-B weights W_s'[t,q]=w1024^{ts} W128^{tq},
    columns packed (q, re/im); s=1 im-chunk sign-folded (operand is -u1im)."""
    if "bw" in _cache:
        return _cache["bw"]
    t = np.arange(128)[:, None]
    tiles = []
    for s in (0, 4):
        qs = np.arange(65) if s == 0 else np.arange(64)
        W = np.exp(-2j * np.pi * t * (8 * qs[None, :] + s) / 1024.0)
        cols = []
        for qi, q in enumerate(qs):
            if s == 0 and q in (0, 64):
                cols.append(W[:, qi].real)
            else:
                cols.append(W[:, qi].real)
                cols.append(W[:, qi].imag)
        tiles.append(np.stack(cols, 1))
    for s in (1, 2, 3):
        for ot in range(2):
            qs = np.arange(64) + 64 * ot
            W = np.exp(-2j * np.pi * t * (8 * qs[None, :] + s) / 1024.0)
            cre, cim = [], []
            for qi in range(64):
                cre.append(W[:, qi].real)
                cre.append(W[:, qi].imag)
                cim.append(-W[:, qi].imag)
                cim.append(W[:, qi].real)
            Wre = np.stack(cre, 1)
            Wim = np.stack(cim, 1)
            if s == 1:
                Wim = -Wim
            tiles.append(Wre)
            tiles.append(Wim)
    bw = np.stack(tiles, 1).astype(np.float32)  # [128, 14, 128]
    _cache["bw"] = bw
    return bw


# stage-B matmul list: (bw_idx, u_name, start, stop) per output stile
_BMMS = [
    [(0, "u0", True, True)],
    [(1, "u4", True, True)],
    [(2, "u1re", True, False), (3, "u1imn", False, True)],
    [(4, "u1re", True, False), (5, "u1imn", False, True)],
    [(6, "u2re", True, False), (7, "u2im", False, True)],
    [(8, "u2re", True, False), (9, "u2im", False, True)],
    [(10, "u3re", True, False), (11, "u3im", False, True)],
    [(12, "u3re", True, False), (13, "u3im", False, True)],
]
_ST_S = [(0, 0), (4, 0), (1, 0), (1, 1), (2, 0), (2, 1), (3, 0), (3, 1)]


def _fft8_maps():
    """Static per-stile unpack maps: row -> (k, comp, sign)."""
    if "maps" in _cache:
        return _cache["maps"]
    maps = []
    for st in range(NST):
        s, ot = _ST_S[st]
        kk = np.zeros(128, np.int64)
        cc = np.zeros(128, np.int64)
        sg = np.zeros(128, np.float32)
        if s == 0:
            row = 0
            for q in range(65):
                if q in (0, 64):
                    kk[row], cc[row], sg[row] = 8 * q, 0, 1.0
                    row += 1
                else:
                    kk[row], cc[row], sg[row] = 8 * q, 0, 1.0
                    kk[row + 1], cc[row + 1], sg[row + 1] = 8 * q, 1, 1.0
                    row += 2
            assert row == 128
        elif s == 4:
            for qi in range(64):
                kk[2 * qi], cc[2 * qi], sg[2 * qi] = 8 * qi + 4, 0, 1.0
                kk[2 * qi + 1], cc[2 * qi + 1], sg[2 * qi + 1] = 8 * qi + 4, 1, 1.0
        else:
            for qi in range(64):
                q = qi + 64 * ot
                k = 8 * q + s
                if k <= 512:
                    kk[2 * qi], cc[2 * qi], sg[2 * qi] = k, 0, 1.0
                    kk[2 * qi + 1], cc[2 * qi + 1], sg[2 * qi + 1] = k, 1, 1.0
                else:
                    km = 1024 - k
                    assert 0 <= km <= 512
                    kk[2 * qi], cc[2 * qi], sg[2 * qi] = km, 0, 1.0
                    kk[2 * qi + 1], cc[2 * qi + 1], sg[2 * qi + 1] = km, 1, -1.0
        maps.append((kk, cc, sg))
    _cache["maps"] = maps
    return maps


def _fft8_structure(dlnf):
    """Per-core chain structure + L-weight blocks (lhsT [n,tau] layout)."""
    L, idx_lo = _build_L_dense(dlnf)
    cores = []
    for c in range(NCORES):
        chains = []  # (dl, I, [J...], blk_base)
        blocks = []
        for dl in range(D_PER):
            d = D_PER * c + dl
            for I in range(8):
                seg = idx_lo[d, 128 * I : 128 * (I + 1)]
                lo = int(seg.min()) // 128
                hi = int(seg.max() + 1) // 128
                js = list(range(lo, hi + 1))
                chains.append((dl, I, js, len(blocks)))
                for J in js:
                    blk = L[d, 128 * I : 128 * (I + 1), 128 * J : 128 * (J + 1)]
                    blocks.append(np.ascontiguousarray(blk.T))  # [n, tau]
        lw = np.stack(blocks, 1).astype(ml_dtypes.bfloat16)  # [128, nblk, 128]
        cores.append({"chains": chains, "lw": lw, "nblk": len(blocks)})
    return cores


def _build_nc_fft8(struct, iters=1):
    import concourse.bacc as bacc
    import concourse.mybir as mybir
    from concourse import tile

    f32 = mybir.dt.float32
    bf16 = mybir.dt.bfloat16
    nblk = struct["nblk"]
    chains = struct["chains"]

    nc = bacc.Bacc("TRN2", target_bir_lowering=False, debug=False)
    xt_d = nc.dram_tensor("xt", [128, B, 4, MQP], bf16, kind="ExternalInput")
    lw_d = nc.dram_tensor("lw", [128, nblk, 128], bf16, kind="ExternalInput")
    bw_d = nc.dram_tensor("bw", [128, NBW, 128], bf16, kind="ExternalInput")
    # out[slice = 4*dl + b, stile, p, w]
    out_d = nc.dram_tensor("out", [8, NST, 128, NWP], bf16, kind="ExternalOutput")

    A = mybir.AluOpType

    def body(nc, tc, pools):
        (xpool, lwpool, rsb, upool, tpool, xst, rps, bps) = pools
        x_sb = xpool.tile([128, B, 4, MQP], bf16, name="x_sb")
        lw_sb = lwpool.tile([128, nblk, 128], bf16, name="lw_sb")
        bw_sb = lwpool.tile([128, NBW, 128], bf16, name="bw_sb")
        nc.sync.dma_start(lw_sb, lw_d[:])
        for b in range(B):
            nc.sync.dma_start(x_sb[:, b], xt_d[:, b])
        nc.sync.dma_start(bw_sb, bw_d[:])

        rtiles = [None, None]  # per pipeline parity: dict I -> sbuf tile
        utiles = [None, None]

        def emit_R(s):
            dl, b = s // 4, s % 4
            rt = {}
            for (cdl, I, js, base) in chains:
                if cdl != dl:
                    continue
                ps = rps.tile([128, NWP], f32, name="rp", tag="rp")
                for ci, J in enumerate(js):
                    nc.tensor.matmul(
                        ps,
                        lw_sb[:, base + ci],
                        x_sb[:, b, J % 4, J // 4 : J // 4 + NWP],
                        start=(ci == 0),
                        stop=(ci == len(js) - 1),
                    )
                rsbt = rsb.tile([128, NWP], bf16, name=f"r{I}", tag=f"r{I}")
                nc.scalar.copy(rsbt, ps)
                rt[I] = rsbt
            rtiles[s % 2] = rt

        def emit_A(s):
            r = rtiles[s % 2]
            t = {
                nm: tpool.tile([128, NWP], bf16, name=nm, tag=nm)
                for nm in (
                    "e0", "e1", "e2", "e3", "o0", "o1", "o2", "o3",
                    "t1", "t2", "d1", "d2",
                )
            }
            u = {
                nm: upool.tile([128, NWP], bf16, name=nm, tag=nm)
                for nm in (
                    "u0", "u4", "u1re", "u1imn", "u2re", "u2im", "u3re", "u3im"
                )
            }
            v = nc.vector
            for j in range(4):
                v.tensor_tensor(t[f"e{j}"], r[j], r[j + 4], op=A.add)
            for j in range(4):
                v.tensor_tensor(t[f"o{j}"], r[j], r[j + 4], op=A.subtract)
            v.tensor_tensor(t["t1"], t["e0"], t["e2"], op=A.add)
            v.tensor_tensor(t["t2"], t["e1"], t["e3"], op=A.add)
            v.tensor_tensor(u["u0"], t["t1"], t["t2"], op=A.add)
            v.tensor_tensor(u["u4"], t["t1"], t["t2"], op=A.subtract)
            v.tensor_tensor(u["u2re"], t["e0"], t["e2"], op=A.subtract)
            v.tensor_tensor(u["u2im"], t["e3"], t["e1"], op=A.subtract)
            v.tensor_tensor(t["d1"], t["o1"], t["o3"], op=A.subtract)
            v.tensor_tensor(t["d2"], t["o1"], t["o3"], op=A.add)
            # u1re = k*d1 + o0 ; u3re = -k*d1 + o0
            v.scalar_tensor_tensor(u["u1re"], t["d1"], SQ2H, t["o0"], A.mult, A.add)
            v.scalar_tensor_tensor(u["u3re"], t["d1"], -SQ2H, t["o0"], A.mult, A.add)
            # u1imn = k*d2 + o2 (= -u1im, sign folded in bw); u3im = -k*d2 + o2
            v.scalar_tensor_tensor(u["u1imn"], t["d2"], SQ2H, t["o2"], A.mult, A.add)
            v.scalar_tensor_tensor(u["u3im"], t["d2"], -SQ2H, t["o2"], A.mult, A.add)
            utiles[s % 2] = u

        def emit_B(s):
            u = utiles[s % 2]
            for st in range(NST):
                ps = bps.tile([128, NWP], f32, name="xp", tag="xp")
                for (bwi, unm, sa, so) in _BMMS[st]:
                    nc.tensor.matmul(
                        ps, bw_sb[:, bwi], u[unm], start=sa, stop=so
                    )
                xs = xst.tile([128, NWP], bf16, name="xs", tag="xs")
                nc.vector.tensor_copy(xs, ps)
                nc.scalar.dma_start(out_d[s, st], xs)

        # Emission order shapes each engine's in-order queue:
        #   PE : R0 R1 [B0 R2] [B1 R3] ... — resample of slice s+2 overlaps
        #        stage-B of slice s, so PE never waits on the DVE.
        #   DVE: A0 A1 ev(B0) A2 ev(B1) ... — A(s+1) is queued BEFORE the
        #        evictions of B(s); evictions never delay the next A block.
        emit_R(0)
        emit_R(1)
        emit_A(0)
        for s in range(8):
            if s + 1 < 8:
                emit_A(s + 1)
            emit_B(s)
            if s + 2 < 8:
                emit_R(s + 2)

    with tile.TileContext(nc) as tc:
        with (
            tc.tile_pool(name="xsb", bufs=2) as xpool,
            tc.tile_pool(name="lwsb", bufs=2) as lwpool,
            tc.tile_pool(name="rsb", bufs=2) as rsb,
            tc.tile_pool(name="usb", bufs=2) as upool,
            tc.tile_pool(name="tsb", bufs=1) as tpool,
            tc.tile_pool(name="xst", bufs=6) as xst,
            tc.tile_pool(name="rps", bufs=4, space="PSUM") as rps,
            tc.tile_pool(name="bps", bufs=4, space="PSUM") as bps,
        ):
            pools = (xpool, lwpool, rsb, upool, tpool, xst, rps, bps)
            if iters > 1:
                with tc.For_i(0, iters, 1):
                    body(nc, tc, pools)
            else:
                body(nc, tc, pools)

    nc.compile()
    return nc


def _get_fft8_ncs(dlnf, iters):
    key = ("fft8", dlnf.tobytes(), iters)
    if key not in _cache:
        if ("fft8s", dlnf.tobytes()) not in _cache:
            _cache[("fft8s", dlnf.tobytes())] = _fft8_structure(dlnf)
        structs = _cache[("fft8s", dlnf.tobytes())]
        _cache[key] = [_build_nc_fft8(s, iters) for s in structs]
    return _cache[key]


def _prep_fft8(x, dlnf):
    x = np.asarray(x, dtype=np.float32)
    dlnf = np.asarray(dlnf, dtype=np.float32)
    skey = ("fft8s", dlnf.tobytes())
    if skey not in _cache:
        _cache[skey] = _fft8_structure(dlnf)
    structs = _cache[skey]
    xt_n = x.reshape(B, MQ, 4, 128).transpose(3, 0, 2, 1)
    xt = np.zeros((128, B, 4, MQP), ml_dtypes.bfloat16)
    xt[:, :, :, :MQ] = xt_n.astype(ml_dtypes.bfloat16)
    xt = np.ascontiguousarray(xt)
    bw = _fft8_bweights().astype(ml_dtypes.bfloat16)
    return [
        {"xt": xt, "lw": np.ascontiguousarray(structs[c]["lw"]), "bw": bw}
        for c in range(NCORES)
    ]


def _assemble_fft8(results):
    maps = _fft8_maps()
    full = np.zeros((B, NW, D, NF, 2), dtype=np.float32)
    for c, r in enumerate(results):
        o = np.asarray(r["out"]).astype(np.float32)  # [8, 8, 128, NWP]
        for dl in range(D_PER):
            d = D_PER * c + dl
            for b in range(B):
                sl = o[4 * dl + b]  # [8, 128, NWP]
                for st in range(NST):
                    kk, cc, sg = maps[st]
                    full[b, :, d, kk, cc] = (
                        sl[st, :, :NW] * sg[:, None]
                    )
    return (
        full.reshape(B, NW, D, NF * 2)
        .view(np.complex64)
        .reshape(B, NW, D, NF)
    )


# --------------------------------------------------------------------------
# device program
# --------------------------------------------------------------------------
def _build_nc(iters=1, sched=None):
    import concourse.bacc as bacc
    import concourse.mybir as mybir
    from concourse import tile

    sched = sched or SCHED
    f32 = mybir.dt.float32
    bf16 = mybir.dt.bfloat16
    mm_dt = mybir.dt.float32r if sched == "base" else bf16

    nc = bacc.Bacc("TRN2", target_bir_lowering=False, debug=False)

    # xt[p, b, r, mq] = x[b, 128*(4*mq + r) + p]  (mq innermost: every
    # matmul moving slice is contiguous)
    xt_d = nc.dram_tensor("xt", [128, B, 4, MQP], mm_dt, kind="ExternalInput")
    # g[p, kc, 1024*d + fe] = G_d[128*kc + p, keep[fe]]
    g_d = nc.dram_tensor("g", [128, KC, D_PER * FE], mm_dt, kind="ExternalInput")
    if sched == "base":
        out_d = nc.dram_tensor(
            "out", [B, NFT // 4, 128, 4, NWP], f32, kind="ExternalOutput"
        )
    else:
        # out[b, fg, p, j, w]: f2e tile ft = 2*fg + j, psum partition p
        out_d = nc.dram_tensor(
            "out", [B, NFT // 2, 128, 2, NWP], bf16, kind="ExternalOutput"
        )

    def body_base(nc, tc, xpool, gpool, spool, ppool):
        x_sb = xpool.tile([128, B, 4, MQP], mm_dt, name="x_sb")
        g_sb = gpool.tile([128, KC, D_PER * FE], mm_dt, name="g_sb")
        for b in range(B):
            nc.sync.dma_start(x_sb[:, b], xt_d[:, b])
        for kc in range(KC):
            nc.sync.dma_start(g_sb[:, kc], g_d[:, kc])

        for b in range(B):
            for ftp in range(NFT // 2):
                st = spool.tile([128, 2, NWP], f32, name="st")
                for jj in range(2):
                    ft = 2 * ftp + jj
                    ps = ppool.tile([128, NWP], f32, name="ps", tag="ps")
                    for h in range(2):
                        for kc in range(KC):
                            q, r = divmod(kc, 4)
                            nc.tensor.matmul(
                                ps[:, h * 256 : (h + 1) * 256],
                                g_sb[:, kc, 128 * ft : 128 * (ft + 1)],
                                x_sb[:, b, r, q + h * 256 : q + h * 256 + 256],
                                start=(kc == 0 and h == 0),
                                stop=(kc == KC - 1 and h == 1),
                            )
                    eng = nc.vector.tensor_copy if ft % 2 == 0 else nc.scalar.copy
                    eng(st[:, jj], ps)
                nc.scalar.dma_start(
                    out_d[b, ftp // 2, :, 2 * (ftp % 2) : 2 * (ftp % 2) + 2], st
                )

    def body_ws(nc, tc, xpool, gpool, spool, ppool):
        x_sb = xpool.tile([128, B, 4, MQP], mm_dt, name="x_sb")
        g_sb = gpool.tile([128, KC, D_PER * FE], mm_dt, name="g_sb")
        nc.sync.dma_start(g_sb[:, 0], g_d[:, 0])
        for b in range(B):
            nc.sync.dma_start(x_sb[:, b], xt_d[:, b])
        for kc in range(1, KC):
            nc.sync.dma_start(g_sb[:, kc], g_d[:, kc])

        st = {}
        for ftp in range(NFT):
            ps = [
                ppool.tile([128, NWP], f32, name=f"ps{b}", tag=f"ps{b}")
                for b in range(B)
            ]
            for kc in range(KC):
                q, r = divmod(kc, 4)
                w_ap = g_sb[:, kc, 128 * ftp : 128 * (ftp + 1)]
                for b in range(B):
                    nc.tensor.matmul(
                        ps[b],
                        w_ap,
                        x_sb[:, b, r, q : q + NWP],
                        start=(kc == 0),
                        stop=(kc == KC - 1),
                    )
            jj = ftp % 2
            if jj == 0:
                for b in range(B):
                    st[b] = spool.tile([128, 2, NWP], bf16, name=f"st{b}")
            for b in range(B):
                eng = nc.vector.tensor_copy if b % 2 == 0 else nc.scalar.copy
                eng(st[b][:, jj], ps[b])
            if jj == 1:
                for b in range(B):
                    nc.scalar.dma_start(out_d[b, ftp // 2], st[b])

    body = body_base if sched == "base" else body_ws

    with tile.TileContext(nc) as tc:
        with (
            tc.tile_pool(name="xsb", bufs=2) as xpool,
            tc.tile_pool(name="gsb", bufs=2) as gpool,
            tc.tile_pool(name="stage", bufs=3) as spool,
            tc.tile_pool(
                name="psum", bufs=8 if sched == "base" else 2, space="PSUM"
            ) as ppool,
        ):
            if iters > 1:
                with tc.For_i(0, iters, 1):
                    body(nc, tc, xpool, gpool, spool, ppool)
            else:
                body(nc, tc, xpool, gpool, spool, ppool)

    nc.compile()
    return nc


def _get_nc(iters=1, sched=None):
    sched = sched or SCHED
    key = ("nc", iters, sched)
    if key not in _cache:
        _cache[key] = _build_nc(iters, sched)
    return _cache[key]


# --------------------------------------------------------------------------
# host prep / assembly
# --------------------------------------------------------------------------
def _prep_arrays(x, dlnf, sched=None):
    """Host prep: G matrices + transposed/sharded device input arrays."""
    sched = sched or SCHED
    dt = np.float32 if sched == "base" else ml_dtypes.bfloat16
    x = np.asarray(x, dtype=np.float32)
    dlnf = np.asarray(dlnf, dtype=np.float32)
    G = _build_G(dlnf)                                     # (16, 1024, 1026)
    xt_n = x.reshape(B, MQ, 4, 128).transpose(3, 0, 2, 1)  # (128, B, 4, MQ)
    xt = np.zeros((128, B, 4, MQP), dt)
    xt[:, :, :, :MQ] = xt_n.astype(dt)
    xt = np.ascontiguousarray(xt)
    Ge = G[:, :, _KEEP]                                    # (16, 1024, 1024)
    g_all = Ge.reshape(D, KC, 128, FE).transpose(2, 1, 0, 3)  # (128,KC,D,FE)
    in_maps = [
        {
            "xt": xt,
            "g": np.ascontiguousarray(
                g_all[:, :, c * D_PER : (c + 1) * D_PER]
                .reshape(128, KC, D_PER * FE)
                .astype(dt)
            ),
        }
        for c in range(NCORES)
    ]
    return in_maps


def _assemble(results, sched=None):
    """per-core out2 -> (B, NW, D, NF) complex64."""
    sched = sched or SCHED
    full = np.zeros((B, NW, D, FW), dtype=np.float32)
    for c, r in enumerate(results):
        o = np.asarray(r["out"]).astype(np.float32)[..., :NW]
        o = o.transpose(0, 4, 1, 3, 2).reshape(B, NW, D_PER, FE)
        for dd in range(D_PER):
            full[:, :, c * D_PER + dd, _KEEP] = o[:, :, dd]
    return full.view(np.complex64).reshape(B, NW, D, NF)


# --------------------------------------------------------------------------
# runner (jitted multi-core executable, cached across kernel() calls)
# --------------------------------------------------------------------------
def _make_sharded(nc, devices=None):
    import jax
    from jax.experimental.shard_map import shard_map
    from jax.sharding import Mesh, PartitionSpec

    from concourse import bass2jax as b2j
    import concourse.mybir as mybir

    b2j.install_neuronx_cc_hook()
    partition_name = nc.partition_id_tensor.name if nc.partition_id_tensor else None

    in_names, out_names, out_avals, zero_outs = [], [], [], []
    for alloc in nc.m.functions[0].allocations:
        if not isinstance(alloc, mybir.MemoryLocationSet):
            continue
        name = alloc.memorylocations[0].name
        if alloc.kind == "ExternalInput":
            if name != partition_name:
                in_names.append(name)
        elif alloc.kind == "ExternalOutput":
            out_names.append(name)
            shape = tuple(alloc.tensor_shape)
            dtype = mybir.dt.np(alloc.dtype)
            out_avals.append(jax.core.ShapedArray(shape, dtype))
            zero_outs.append(np.zeros(shape, dtype))
    all_names = in_names + out_names
    if partition_name is not None:
        all_names = all_names + [partition_name]

    def _body(*args):
        operands = list(args)
        if partition_name is not None:
            operands.append(b2j.partition_id_tensor())
        outs = b2j._bass_exec_p.bind(
            *operands,
            out_avals=tuple(out_avals),
            in_names=tuple(all_names),
            out_names=tuple(out_names),
            lowering_input_output_aliases=(),
            sim_require_finite=True,
            sim_require_nnan=True,
            nc=nc,
        )
        return tuple(outs)

    if devices is None:
        devices = jax.devices()[:NCORES]
    mesh = Mesh(np.asarray(devices), ("core",))
    nin = len(in_names) + len(zero_outs)
    sharded = jax.jit(
        shard_map(
            _body,
            mesh=mesh,
            in_specs=(PartitionSpec("core"),) * nin,
            out_specs=(PartitionSpec("core"),) * len(out_names),
            check_rep=False,
        ),
        keep_unused=True,
    )
    return sharded, in_names, out_names, out_avals, zero_outs


def _get_runner(iters, sched=None):
    sched = sched or SCHED
    key = ("runner", iters, sched)
    if key in _cache:
        return _cache[key]

    import jax

    nc = _get_nc(iters, sched)
    sharded, in_names, out_names, out_avals, zero_outs = _make_sharded(nc)

    def call(in_maps):
        concat_in = [
            np.concatenate([in_maps[c][name] for c in range(NCORES)], axis=0)
            for name in in_names
        ] + [
            np.zeros((NCORES * z.shape[0], *z.shape[1:]), z.dtype)
            for z in zero_outs
        ]
        out_arrs = sharded(*concat_in)
        jax.block_until_ready(out_arrs)
        return [
            {
                name: np.asarray(out_arrs[i]).reshape(
                    NCORES, *out_avals[i].shape
                )[c]
                for i, name in enumerate(out_names)
            }
            for c in range(NCORES)
        ]

    _cache[key] = call
    return call


def _get_fft8_runner(dlnf, iters):
    """Heterogeneous per-core programs: 8 single-device executables."""
    key = ("fft8run", dlnf.tobytes(), iters)
    if key in _cache:
        return _cache[key]

    import jax

    ncs = _get_fft8_ncs(dlnf, iters)
    devices = jax.devices()[:NCORES]
    cores = []
    for c in range(NCORES):
        sharded, in_names, out_names, out_avals, zero_outs = _make_sharded(
            ncs[c], devices=[devices[c]]
        )
        cores.append((sharded, in_names, out_names, out_avals, zero_outs))

    def call(in_maps):
        outs = []
        for c in range(NCORES):
            sharded, in_names, out_names, out_avals, zero_outs = cores[c]
            args = [
                jax.device_put(in_maps[c][n], devices[c]) for n in in_names
            ] + [
                jax.device_put(np.zeros(z.shape, z.dtype), devices[c])
                for z in zero_outs
            ]
            outs.append(sharded(*args))
        jax.block_until_ready(outs)
        results = []
        for c in range(NCORES):
            _, _, out_names, out_avals, _ = cores[c]
            results.append(
                {
                    n: np.asarray(outs[c][i]).reshape(out_avals[i].shape)
                    for i, n in enumerate(out_names)
                }
            )
        return results

    _cache[key] = call
    return call


def kernel(x, dlnf, n_hann_splits=1, **_unused):
    iters = int(os.environ.get("KERNEL_ITERS", "1"))
    dlnf32 = np.asarray(dlnf, dtype=np.float32)
    if SCHED == "fft8":
        try:
            in_maps = _prep_fft8(x, dlnf32)
            call = _get_fft8_runner(dlnf32, iters)
            return _assemble_fft8(call(in_maps))
        except Exception:
            import traceback

            traceback.print_exc()
            # fall through to the dense-G path
    in_maps = _prep_arrays(x, dlnf, sched="bf16ws")
    try:
        call = _get_runner(iters, sched="bf16ws")
        results = call(in_maps)
    except Exception:
        # robust fallback: the reference implementation of the SPMD runner
        from concourse.bass_utils import run_bass_kernel_spmd

        nc = _get_nc(iters, sched="bf16ws")
        res = run_bass_kernel_spmd(nc, in_maps, core_ids=list(range(NCORES)))
        results = res.results
    return _assemble(results, sched="bf16ws")


# --------------------------------------------------------------------------
# benchmarking: jit once, time repeated executions (no retrace/relower)
# --------------------------------------------------------------------------
def prepare_bench(x, dlnf, iters, sched=None):
    """Returns run() -> wall seconds for one execution of the iters-body NEFF."""
    import time

    import jax

    sched = sched or SCHED
    if sched == "fft8":
        dlnf32 = np.asarray(dlnf, dtype=np.float32)
        in_maps = _prep_fft8(x, dlnf32)
        ncs = _get_fft8_ncs(dlnf32, iters)
        devices = jax.devices()[:NCORES]
        cores = []
        for c in range(NCORES):
            sharded, in_names, out_names, out_avals, zero_outs = _make_sharded(
                ncs[c], devices=[devices[c]]
            )
            args = [
                jax.device_put(in_maps[c][n], devices[c]) for n in in_names
            ] + [
                jax.device_put(np.zeros(z.shape, z.dtype), devices[c])
                for z in zero_outs
            ]
            cores.append((sharded, args))
        outs = [s(*a) for s, a in cores]
        jax.block_until_ready(outs)

        def run():
            t0 = time.perf_counter()
            o = [s(*a) for s, a in cores]
            jax.block_until_ready(o)
            return time.perf_counter() - t0

        return run
    in_maps = _prep_arrays(x, dlnf, sched)
    nc = _get_nc(iters, sched)
    sharded, in_names, out_names, out_avals, zero_outs = _make_sharded(nc)
    concat_in = [
        np.concatenate([in_maps[c][name] for c in range(NCORES)], axis=0)
        for name in in_names
    ] + [np.zeros((NCORES * z.shape[0], *z.shape[1:]), z.dtype) for z in zero_outs]
    concat_in = [jax.device_put(a) for a in concat_in]

    out = sharded(*concat_in)
    jax.block_until_ready(out)

    def run():
        t0 = time.perf_counter()
        o = sharded(*concat_in)
        jax.block_until_ready(o)
        return time.perf_counter() - t0

    return run


if __name__ == "__main__":
    rng = np.random.default_rng(0)
    x = rng.standard_normal((B, N), dtype=np.float32)
    dlnf = rng.uniform(-0.5, 0.5, size=(D,)).astype(np.float32)
    out = kernel(x, dlnf, 1)
    print("out:", out.shape, out.dtype)


# revision 27
# speedup vs baseline: 1.2689x; 1.0589x over previous
"""DechirpSTFT Trainium2 kernel.

The PE on these cores is purely column-rate limited (~1.8-2.0 cols/ns;
instruction count and weight reloads are free), so the win comes from
reducing matmul column-passes.  Default schedule 'fft8' factorizes the
per-chirp operator (hann window -> lerp resample * jac -> rfft(1024))
instead of applying it as one dense 1024x1026 matrix:

  R (PE) : r = L_d @ xw      banded resample blocks; i(tau) is a monotone
           bijection so the band sum is ~15 chunk-passes/chirp vs 64 dense
  A (DVE): u_s[t] = sum_j r[t+128j] w8^{js}   radix-8 combine, 20 tensor
           ops/slice, Hermitian-packed (s=0..4); twiddles w1024^{ts} and
           the u1im sign are folded into the stage-B weights
  B (PE) : X[8q+s] = sum_t u_s[t] W_s'[t,q]   per-residue DFT-128, col-
           packed 128-real out tiles; host conj-mirrors fill s=5,6,7

Column-passes drop 262144 -> ~119K per core.  All-bf16 intermediates:
rel err 4.4e-3 vs the 2e-2 gate (fp8 e4m3 measured 2.7-3.7e-2, fails).
The band structure depends on dlnf, so per-core programs are compiled
per call (cached by dlnf bytes) and run as 8 single-device executables.

Scheduling details that mattered (all measured on HW):
  * DVE may read ONE PSUM operand: level-1 butterflies read r0-r3 from
    PSUM directly; r4-r7 are copied to SBUF by ACT.  Chains issue
    pairwise (r0,r4,r1,r5,..) so banks free early.
  * Engine queues are in-order: A(s+1) is emitted BEFORE B(s)'s
    evictions, and R(s+2) before B(s), so neither PE nor DVE ever waits
    a full stage.  X evictions split 3/5 between DVE and ACT.
  * fp8 DoubleRow, multi-bank matmul outputs (>512 cols), and DMA from
    PSUM are all rejected by HW/walrus.

Fallback schedule 'bf16ws' (dense G, bf16, weight-stationary, ~1.25x
slower) runs if anything in the fft8 path throws; 'base' is the original
fp32r variant, selectable via KSCHED for A/B.

Sharding: D=16 chirp rates, 2 per core across 8 cores (x replicated).
No cross-core communication.

Timing: no NTFF profiling is reachable through this axon client, so
test.py measures HW time as (wall(For_i(T)) - wall(For_i(1))) / (T-1)
with the jitted executable and device-resident inputs held across calls.
"""

import os
import sys

sys.path.insert(0, "/opt/trn_rl_repo")

import numpy as np
import ml_dtypes

# ---- problem constants (hardcoded; kernel.py must be self-contained) ----
B = 4
N = 262144
K = 1024
HOP = 512
NW = (N - K) // HOP + 1          # 511
KTAU = 1024
NF = 513                         # rfft bins
FW = 2 * NF                      # 1026 interleaved re/im
FE = 1024                        # live f2 cols per chirp (im0/imNyq dropped)
NWP = 512                        # padded window count
MQP = 513                        # padded m-quads so window 511 stays in-bounds
D = 16
NCORES = 8
D_PER = D // NCORES              # 2
KC = 8                           # contraction chunks of 128
MQ = N // 512                    # 512
NFT = 16                         # f2e weight tiles of 128 per core
EPS = 1e-8

SCHED = os.environ.get("KSCHED", "fft8")

_cache = {}


# --------------------------------------------------------------------------
# host-side G construction
# --------------------------------------------------------------------------
def _build_tables_np(dlnf):
    """Reference's per-chirp tables in numpy float32 (fallback path)."""
    dlnf = dlnf.astype(np.float32)
    beta = (2.0 * dlnf).astype(np.float32)
    small = np.abs(beta) < EPS
    beta_safe = np.where(small, np.float32(EPS), beta).astype(np.float32)
    e2b = np.exp(2.0 * beta_safe).astype(np.float32)

    tau = (2.0 * np.arange(KTAU, dtype=np.float32) / KTAU - 1.0).astype(np.float32)
    t_source = np.log(
        1.0 + (tau[None, :] + 1.0) / 2.0 * (e2b[:, None] - 1.0)
    ).astype(np.float32)
    t_source = (t_source / beta_safe[:, None] - 1.0).astype(np.float32)
    t_source = np.where(small[:, None], tau[None, :], t_source)

    tau_mid = np.float32(2.0 * (KTAU // 2) / KTAU - 1.0)
    t_mid = (
        np.log(1.0 + (tau_mid + 1.0) / 2.0 * (e2b - 1.0)) / beta_safe - 1.0
    ).astype(np.float32)
    t_mid = np.where(small, tau_mid, t_mid)

    jac = np.exp(-beta_safe[:, None] * (t_source - t_mid[:, None])).astype(np.float32)
    jac = np.where(small[:, None], np.float32(1.0), jac)

    idx = (np.float32(K / 2.0) * (t_source + 1.0)).astype(np.float32)
    idx_lo = np.clip(idx.astype(np.int32), 0, K - 2)
    frac = (idx - idx_lo.astype(np.float32)).astype(np.float32)
    return idx_lo, frac, jac


def _build_tables(dlnf):
    """Per-chirp tables, computed with jax on the CPU backend so the f32
    transcendentals (log/exp) match the reference bit-for-bit."""
    try:
        import jax
        import jax.numpy as jnp

        cpu = jax.devices("cpu")[0]
    except Exception:
        return _build_tables_np(dlnf)

    with jax.default_device(cpu):
        beta = 2.0 * jnp.asarray(dlnf, dtype=jnp.float32)
        small = jnp.abs(beta) < EPS
        beta_safe = jnp.where(small, EPS, beta)
        e2b = jnp.exp(2.0 * beta_safe)

        tau = 2.0 * jnp.arange(KTAU, dtype=jnp.float32) / KTAU - 1.0
        t_source = (
            jnp.log(1.0 + (tau[None, :] + 1.0) / 2.0 * (e2b[:, None] - 1.0))
            / beta_safe[:, None]
            - 1.0
        )
        t_source = jnp.where(small[:, None], tau[None, :], t_source)

        tau_mid = 2.0 * (KTAU // 2) / KTAU - 1.0
        t_mid = (
            jnp.log(1.0 + (tau_mid + 1.0) / 2.0 * (e2b - 1.0)) / beta_safe - 1.0
        )
        t_mid = jnp.where(small, tau_mid, t_mid)

        jac = jnp.exp(-beta_safe[:, None] * (t_source - t_mid[:, None]))
        jac = jnp.where(small[:, None], 1.0, jac)

        idx = (K / 2.0) * (t_source + 1.0)
        idx_lo = jnp.clip(idx.astype(jnp.int32), 0, K - 2)
        frac = idx - idx_lo.astype(jnp.float32)
    return np.asarray(idx_lo), np.asarray(frac), np.asarray(jac)


def _build_G(dlnf):
    """G[d, k, f2] f32: fused hann * lerp-resample * jac * rfft operator."""
    nd = dlnf.shape[0]
    idx_lo, frac, jac = _build_tables(dlnf)
    t = np.arange(KTAU, dtype=np.float64)
    f = np.arange(NF, dtype=np.float64)
    ang = 2.0 * np.pi * np.outer(t, f) / KTAU
    Wre = np.cos(ang)
    Wim = -np.sin(ang)
    n = np.arange(K, dtype=np.float32)
    hann = (0.5 * (1.0 - np.cos(2.0 * np.pi * n / K))).astype(np.float32)

    G = np.zeros((nd, K, FW), dtype=np.float64)
    for d in range(nd):
        c_lo = (jac[d] * (1.0 - frac[d])).astype(np.float64)
        c_hi = (jac[d] * frac[d]).astype(np.float64)
        Gre = np.zeros((K, NF))
        Gim = np.zeros((K, NF))
        np.add.at(Gre, idx_lo[d], c_lo[:, None] * Wre)
        np.add.at(Gim, idx_lo[d], c_lo[:, None] * Wim)
        np.add.at(Gre, idx_lo[d] + 1, c_hi[:, None] * Wre)
        np.add.at(Gim, idx_lo[d] + 1, c_hi[:, None] * Wim)
        G[d, :, 0::2] = Gre
        G[d, :, 1::2] = Gim
    G *= hann[None, :, None].astype(np.float64)
    return G.astype(np.float32)


# live G columns: drop im(bin0) (col 1) and im(Nyquist) (col 1025)
_KEEP = np.concatenate(([0], np.arange(2, 1025)))


# --------------------------------------------------------------------------
# fft8: factorized pipeline  r = L_d @ xw  ->  radix-8 combine (DVE)
#   -> per-residue folded DFT-128 (PE).  ~119K PE column-passes vs 262K
# for the dense-G matmul.  Math validated in mock_fft8.py (bf16 rel err
# 3.2e-3 vs the 2e-2 gate).
# --------------------------------------------------------------------------
NST = 8          # output stiles per slice: [s0, s4, s1t0, s1t1, s2t0.., s3t1]
NBW = 14         # stage-B weight tiles
SQ2H = float(np.sqrt(2.0) / 2.0)


def _build_L_dense(dlnf):
    """L[d, tau, n] f32 resample operator with hann folded."""
    idx_lo, frac, jac = _build_tables(dlnf)
    n = np.arange(K, dtype=np.float32)
    hann = (0.5 * (1.0 - np.cos(2.0 * np.pi * n / K))).astype(np.float32)
    nd = dlnf.shape[0]
    L = np.zeros((nd, K, K), dtype=np.float32)
    rows = np.arange(K)
    for d in range(nd):
        c_lo = jac[d] * (1.0 - frac[d])
        c_hi = jac[d] * frac[d]
        L[d, rows, idx_lo[d]] += c_lo * hann[idx_lo[d]]
        L[d, rows, idx_lo[d] + 1] += c_hi * hann[idx_lo[d] + 1]
    return L, idx_lo


def _fft8_bweights():
    """bw [128, 14, 128] f32: stage-B weights W_s'[t,q]=w1024^{ts} W128^{tq},
    columns packed (q, re/im); s=1 im-chunk sign-folded (operand is -u1im)."""
    if "bw" in _cache:
        return _cache["bw"]
    t = np.arange(128)[:, None]
    tiles = []
    for s in (0, 4):
        qs = np.arange(65) if s == 0 else np.arange(64)
        W = np.exp(-2j * np.pi * t * (8 * qs[None, :] + s) / 1024.0)
        cols = []
        for qi, q in enumerate(qs):
            if s == 0 and q in (0, 64):
                cols.append(W[:, qi].real)
            else:
                cols.append(W[:, qi].real)
                cols.append(W[:, qi].imag)
        tiles.append(np.stack(cols, 1))
    for s in (1, 2, 3):
        for ot in range(2):
            qs = np.arange(64) + 64 * ot
            W = np.exp(-2j * np.pi * t * (8 * qs[None, :] + s) / 1024.0)
            cre, cim = [], []
            for qi in range(64):
                cre.append(W[:, qi].real)
                cre.append(W[:, qi].imag)
                cim.append(-W[:, qi].imag)
                cim.append(W[:, qi].real)
            Wre = np.stack(cre, 1)
            Wim = np.stack(cim, 1)
            if s == 1:
                Wim = -Wim
            tiles.append(Wre)
            tiles.append(Wim)
    bw = np.stack(tiles, 1).astype(np.float32)  # [128, 14, 128]
    _cache["bw"] = bw
    return bw


# stage-B matmul list: (bw_idx, u_name, start, stop) per output stile
_BMMS = [
    [(0, "u0", True, True)],
    [(1, "u4", True, True)],
    [(2, "u1re", True, False), (3, "u1imn", False, True)],
    [(4, "u1re", True, False), (5, "u1imn", False, True)],
    [(6, "u2re", True, False), (7, "u2im", False, True)],
    [(8, "u2re", True, False), (9, "u2im", False, True)],
    [(10, "u3re", True, False), (11, "u3im", False, True)],
    [(12, "u3re", True, False), (13, "u3im", False, True)],
]
_ST_S = [(0, 0), (4, 0), (1, 0), (1, 1), (2, 0), (2, 1), (3, 0), (3, 1)]


def _fft8_maps():
    """Static per-stile unpack maps: row -> (k, comp, sign)."""
    if "maps" in _cache:
        return _cache["maps"]
    maps = []
    for st in range(NST):
        s, ot = _ST_S[st]
        kk = np.zeros(128, np.int64)
        cc = np.zeros(128, np.int64)
        sg = np.zeros(128, np.float32)
        if s == 0:
            row = 0
            for q in range(65):
                if q in (0, 64):
                    kk[row], cc[row], sg[row] = 8 * q, 0, 1.0
                    row += 1
                else:
                    kk[row], cc[row], sg[row] = 8 * q, 0, 1.0
                    kk[row + 1], cc[row + 1], sg[row + 1] = 8 * q, 1, 1.0
                    row += 2
            assert row == 128
        elif s == 4:
            for qi in range(64):
                kk[2 * qi], cc[2 * qi], sg[2 * qi] = 8 * qi + 4, 0, 1.0
                kk[2 * qi + 1], cc[2 * qi + 1], sg[2 * qi + 1] = 8 * qi + 4, 1, 1.0
        else:
            for qi in range(64):
                q = qi + 64 * ot
                k = 8 * q + s
                if k <= 512:
                    kk[2 * qi], cc[2 * qi], sg[2 * qi] = k, 0, 1.0
                    kk[2 * qi + 1], cc[2 * qi + 1], sg[2 * qi + 1] = k, 1, 1.0
                else:
                    km = 1024 - k
                    assert 0 <= km <= 512
                    kk[2 * qi], cc[2 * qi], sg[2 * qi] = km, 0, 1.0
                    kk[2 * qi + 1], cc[2 * qi + 1], sg[2 * qi + 1] = km, 1, -1.0
        maps.append((kk, cc, sg))
    _cache["maps"] = maps
    return maps


def _fft8_structure(dlnf):
    """Per-core chain structure + L-weight blocks (lhsT [n,tau] layout)."""
    L, idx_lo = _build_L_dense(dlnf)
    cores = []
    for c in range(NCORES):
        chains = []  # (dl, I, [J...], blk_base)
        blocks = []
        for dl in range(D_PER):
            d = D_PER * c + dl
            for I in range(8):
                seg = idx_lo[d, 128 * I : 128 * (I + 1)]
                lo = int(seg.min()) // 128
                hi = int(seg.max() + 1) // 128
                js = list(range(lo, hi + 1))
                chains.append((dl, I, js, len(blocks)))
                for J in js:
                    blk = L[d, 128 * I : 128 * (I + 1), 128 * J : 128 * (J + 1)]
                    blocks.append(np.ascontiguousarray(blk.T))  # [n, tau]
        lw = np.stack(blocks, 1).astype(ml_dtypes.bfloat16)  # [128, nblk, 128]
        cores.append({"chains": chains, "lw": lw, "nblk": len(blocks)})
    return cores


def _build_nc_fft8(struct, iters=1):
    import concourse.bacc as bacc
    import concourse.mybir as mybir
    from concourse import tile

    f32 = mybir.dt.float32
    bf16 = mybir.dt.bfloat16
    nblk = struct["nblk"]
    chains = struct["chains"]

    nc = bacc.Bacc("TRN2", target_bir_lowering=False, debug=False)
    xt_d = nc.dram_tensor("xt", [128, B, 4, MQP], bf16, kind="ExternalInput")
    lw_d = nc.dram_tensor("lw", [128, nblk, 128], bf16, kind="ExternalInput")
    bw_d = nc.dram_tensor("bw", [128, NBW, 128], bf16, kind="ExternalInput")
    # out[slice = 4*dl + b, stile, p, w]
    out_d = nc.dram_tensor("out", [8, NST, 128, NWP], bf16, kind="ExternalOutput")

    A = mybir.AluOpType

    def body(nc, tc, pools):
        (xpool, lwpool, rsb, upool, tpool, xst, rps, rss, bps) = pools
        x_sb = xpool.tile([128, B, 4, MQP], bf16, name="x_sb")
        lw_sb = lwpool.tile([128, nblk, 128], bf16, name="lw_sb")
        bw_sb = lwpool.tile([128, NBW, 128], bf16, name="bw_sb")
        nc.sync.dma_start(lw_sb, lw_d[:])
        for b in range(B):
            nc.sync.dma_start(x_sb[:, b], xt_d[:, b])
        nc.sync.dma_start(bw_sb, bw_d[:])

        rtiles = [None, None]  # per pipeline parity: dict I -> sbuf tile
        utiles = [None, None]

        def emit_R(s):
            dl, b = s // 4, s % 4
            ch = {I: (js, base) for (cdl, I, js, base) in chains if cdl == dl}

            def chain(ps_half, I):
                js, base = ch[I]
                for ci, J in enumerate(js):
                    nc.tensor.matmul(
                        ps_half,
                        lw_sb[:, base + ci],
                        x_sb[:, b, J % 4, J // 4 : J // 4 + NWP],
                        start=(ci == 0),
                        stop=(ci == len(js) - 1),
                    )

            # 2-bank PSUM pairs coarsen the sync grain: one DVE butterfly op
            # and one ACT copy per PAIR instead of per tile.  Longs [r0|r1],
            # [r2|r3] stay in PSUM (the one PSUM operand DVE allows) on a
            # 2-deep rotation; shorts [r4|r5], [r6|r7] are copied to SBUF.
            L01 = rps.tile([128, 2, NWP], f32, name="L01", tag="rp")
            S45p = rss.tile([128, 2, NWP], f32, name="S45p", tag="rq")
            chain(L01[:, 0], 0)
            chain(S45p[:, 0], 4)
            chain(L01[:, 1], 1)
            chain(S45p[:, 1], 5)
            S45 = rsb.tile([128, 2, NWP], bf16, name="s45", tag="s45")
            nc.scalar.copy(S45, S45p)
            L23 = rps.tile([128, 2, NWP], f32, name="L23", tag="rp")
            S67p = rss.tile([128, 2, NWP], f32, name="S67p", tag="rq")
            chain(L23[:, 0], 2)
            chain(S67p[:, 0], 6)
            chain(L23[:, 1], 3)
            chain(S67p[:, 1], 7)
            S67 = rsb.tile([128, 2, NWP], bf16, name="s67", tag="s67")
            nc.scalar.copy(S67, S67p)
            rtiles[s % 2] = (L01, L23, S45, S67)

        def emit_A(s):
            L01, L23, S45, S67 = rtiles[s % 2]
            tw = {
                nm: tpool.tile([128, 2, NWP], bf16, name=nm, tag=nm)
                for nm in ("e01", "e23", "o01", "o23", "t12")
            }
            d1 = tpool.tile([128, NWP], bf16, name="d1", tag="d1")
            d2 = tpool.tile([128, NWP], bf16, name="d2", tag="d2")
            u = {
                nm: upool.tile([128, NWP], bf16, name=nm, tag=nm)
                for nm in (
                    "u0", "u4", "u1re", "u1imn", "u2re", "u2im", "u3re", "u3im"
                )
            }
            v = nc.vector
            # level-1 butterflies, double-width: e01=[e0|e1]=[r0+r4|r1+r5]..
            v.tensor_tensor(tw["e01"], L01, S45, op=A.add)
            v.tensor_tensor(tw["o01"], L01, S45, op=A.subtract)
            v.tensor_tensor(tw["e23"], L23, S67, op=A.add)
            v.tensor_tensor(tw["o23"], L23, S67, op=A.subtract)
            v.tensor_tensor(tw["t12"], tw["e01"], tw["e23"], op=A.add)  # [t1|t2]
            v.tensor_tensor(u["u0"], tw["t12"][:, 0], tw["t12"][:, 1], op=A.add)
            v.tensor_tensor(u["u4"], tw["t12"][:, 0], tw["t12"][:, 1], op=A.subtract)
            v.tensor_tensor(u["u2re"], tw["e01"][:, 0], tw["e23"][:, 0], op=A.subtract)
            v.tensor_tensor(u["u2im"], tw["e23"][:, 1], tw["e01"][:, 1], op=A.subtract)
            v.tensor_tensor(d1, tw["o01"][:, 1], tw["o23"][:, 1], op=A.subtract)
            v.tensor_tensor(d2, tw["o01"][:, 1], tw["o23"][:, 1], op=A.add)
            # u1re = k*d1 + o0 ; u3re = -k*d1 + o0   (o0 = o01[:,0])
            v.scalar_tensor_tensor(u["u1re"], d1, SQ2H, tw["o01"][:, 0], A.mult, A.add)
            v.scalar_tensor_tensor(u["u3re"], d1, -SQ2H, tw["o01"][:, 0], A.mult, A.add)
            # u1imn = k*d2 + o2 (sign folded in bw); u3im = -k*d2 + o2
            v.scalar_tensor_tensor(u["u1imn"], d2, SQ2H, tw["o23"][:, 0], A.mult, A.add)
            v.scalar_tensor_tensor(u["u3im"], d2, -SQ2H, tw["o23"][:, 0], A.mult, A.add)
            utiles[s % 2] = u

        def emit_B(s):
            u = utiles[s % 2]
            for st in range(NST):
                ps = bps.tile([128, NWP], f32, name="xp", tag="xp")
                for (bwi, unm, sa, so) in _BMMS[st]:
                    nc.tensor.matmul(
                        ps, bw_sb[:, bwi], u[unm], start=sa, stop=so
                    )
                xs = xst.tile([128, NWP], bf16, name="xs", tag="xs")
                # split evictions: 3 on DVE, 5 on ACT (both stay under the
                # PE's per-slice budget)
                if st < 3:
                    nc.vector.tensor_copy(xs, ps)
                else:
                    nc.scalar.copy(xs, ps)
                nc.scalar.dma_start(out_d[s, st], xs)

        # Emission order shapes each engine's in-order queue:
        #   PE : R0 R1 [B0 R2] [B1 R3] ... — resample of slice s+2 overlaps
        #        stage-B of slice s, so PE never waits on the DVE.
        #   DVE: A0 A1 ev(B0) A2 ev(B1) ... — A(s+1) is queued BEFORE the
        #        evictions of B(s); evictions never delay the next A block.
        emit_R(0)
        emit_R(1)
        emit_A(0)
        for s in range(8):
            if s + 1 < 8:
                emit_A(s + 1)
            emit_B(s)
            if s + 2 < 8:
                emit_R(s + 2)

    with tile.TileContext(nc) as tc:
        with (
            tc.tile_pool(name="xsb", bufs=2) as xpool,
            tc.tile_pool(name="lwsb", bufs=2) as lwpool,
            tc.tile_pool(name="rsb", bufs=2) as rsb,
            tc.tile_pool(name="usb", bufs=2) as upool,
            tc.tile_pool(name="tsb", bufs=1) as tpool,
            tc.tile_pool(name="xst", bufs=6) as xst,
            tc.tile_pool(name="rps", bufs=2, space="PSUM") as rps,
            tc.tile_pool(name="rss", bufs=1, space="PSUM") as rss,
            tc.tile_pool(name="bps", bufs=2, space="PSUM") as bps,
        ):
            pools = (xpool, lwpool, rsb, upool, tpool, xst, rps, rss, bps)
            if iters > 1:
                with tc.For_i(0, iters, 1):
                    body(nc, tc, pools)
            else:
                body(nc, tc, pools)

    nc.compile()
    return nc


def _get_fft8_ncs(dlnf, iters):
    key = ("fft8", dlnf.tobytes(), iters)
    if key not in _cache:
        if ("fft8s", dlnf.tobytes()) not in _cache:
            _cache[("fft8s", dlnf.tobytes())] = _fft8_structure(dlnf)
        structs = _cache[("fft8s", dlnf.tobytes())]
        _cache[key] = [_build_nc_fft8(s, iters) for s in structs]
    return _cache[key]


def _prep_fft8(x, dlnf):
    x = np.asarray(x, dtype=np.float32)
    dlnf = np.asarray(dlnf, dtype=np.float32)
    skey = ("fft8s", dlnf.tobytes())
    if skey not in _cache:
        _cache[skey] = _fft8_structure(dlnf)
    structs = _cache[skey]
    xt_n = x.reshape(B, MQ, 4, 128).transpose(3, 0, 2, 1)
    xt = np.zeros((128, B, 4, MQP), ml_dtypes.bfloat16)
    xt[:, :, :, :MQ] = xt_n.astype(ml_dtypes.bfloat16)
    xt = np.ascontiguousarray(xt)
    bw = _fft8_bweights().astype(ml_dtypes.bfloat16)
    return [
        {"xt": xt, "lw": np.ascontiguousarray(structs[c]["lw"]), "bw": bw}
        for c in range(NCORES)
    ]


def _assemble_fft8(results):
    maps = _fft8_maps()
    full = np.zeros((B, NW, D, NF, 2), dtype=np.float32)
    for c, r in enumerate(results):
        o = np.asarray(r["out"]).astype(np.float32)  # [8, 8, 128, NWP]
        for dl in range(D_PER):
            d = D_PER * c + dl
            for b in range(B):
                sl = o[4 * dl + b]  # [8, 128, NWP]
                for st in range(NST):
                    kk, cc, sg = maps[st]
                    full[b, :, d, kk, cc] = (
                        sl[st, :, :NW] * sg[:, None]
                    )
    return (
        full.reshape(B, NW, D, NF * 2)
        .view(np.complex64)
        .reshape(B, NW, D, NF)
    )


# --------------------------------------------------------------------------
# device program
# --------------------------------------------------------------------------
def _build_nc(iters=1, sched=None):
    import concourse.bacc as bacc
    import concourse.mybir as mybir
    from concourse import tile

    sched = sched or SCHED
    f32 = mybir.dt.float32
    bf16 = mybir.dt.bfloat16
    mm_dt = mybir.dt.float32r if sched == "base" else bf16

    nc = bacc.Bacc("TRN2", target_bir_lowering=False, debug=False)

    # xt[p, b, r, mq] = x[b, 128*(4*mq + r) + p]  (mq innermost: every
    # matmul moving slice is contiguous)
    xt_d = nc.dram_tensor("xt", [128, B, 4, MQP], mm_dt, kind="ExternalInput")
    # g[p, kc, 1024*d + fe] = G_d[128*kc + p, keep[fe]]
    g_d = nc.dram_tensor("g", [128, KC, D_PER * FE], mm_dt, kind="ExternalInput")
    if sched == "base":
        out_d = nc.dram_tensor(
            "out", [B, NFT // 4, 128, 4, NWP], f32, kind="ExternalOutput"
        )
    else:
        # out[b, fg, p, j, w]: f2e tile ft = 2*fg + j, psum partition p
        out_d = nc.dram_tensor(
            "out", [B, NFT // 2, 128, 2, NWP], bf16, kind="ExternalOutput"
        )

    def body_base(nc, tc, xpool, gpool, spool, ppool):
        x_sb = xpool.tile([128, B, 4, MQP], mm_dt, name="x_sb")
        g_sb = gpool.tile([128, KC, D_PER * FE], mm_dt, name="g_sb")
        for b in range(B):
            nc.sync.dma_start(x_sb[:, b], xt_d[:, b])
        for kc in range(KC):
            nc.sync.dma_start(g_sb[:, kc], g_d[:, kc])

        for b in range(B):
            for ftp in range(NFT // 2):
                st = spool.tile([128, 2, NWP], f32, name="st")
                for jj in range(2):
                    ft = 2 * ftp + jj
                    ps = ppool.tile([128, NWP], f32, name="ps", tag="ps")
                    for h in range(2):
                        for kc in range(KC):
                            q, r = divmod(kc, 4)
                            nc.tensor.matmul(
                                ps[:, h * 256 : (h + 1) * 256],
                                g_sb[:, kc, 128 * ft : 128 * (ft + 1)],
                                x_sb[:, b, r, q + h * 256 : q + h * 256 + 256],
                                start=(kc == 0 and h == 0),
                                stop=(kc == KC - 1 and h == 1),
                            )
                    eng = nc.vector.tensor_copy if ft % 2 == 0 else nc.scalar.copy
                    eng(st[:, jj], ps)
                nc.scalar.dma_start(
                    out_d[b, ftp // 2, :, 2 * (ftp % 2) : 2 * (ftp % 2) + 2], st
                )

    def body_ws(nc, tc, xpool, gpool, spool, ppool):
        x_sb = xpool.tile([128, B, 4, MQP], mm_dt, name="x_sb")
        g_sb = gpool.tile([128, KC, D_PER * FE], mm_dt, name="g_sb")
        nc.sync.dma_start(g_sb[:, 0], g_d[:, 0])
        for b in range(B):
            nc.sync.dma_start(x_sb[:, b], xt_d[:, b])
        for kc in range(1, KC):
            nc.sync.dma_start(g_sb[:, kc], g_d[:, kc])

        st = {}
        for ftp in range(NFT):
            ps = [
                ppool.tile([128, NWP], f32, name=f"ps{b}", tag=f"ps{b}")
                for b in range(B)
            ]
            for kc in range(KC):
                q, r = divmod(kc, 4)
                w_ap = g_sb[:, kc, 128 * ftp : 128 * (ftp + 1)]
                for b in range(B):
                    nc.tensor.matmul(
                        ps[b],
                        w_ap,
                        x_sb[:, b, r, q : q + NWP],
                        start=(kc == 0),
                        stop=(kc == KC - 1),
                    )
            jj = ftp % 2
            if jj == 0:
                for b in range(B):
                    st[b] = spool.tile([128, 2, NWP], bf16, name=f"st{b}")
            for b in range(B):
                eng = nc.vector.tensor_copy if b % 2 == 0 else nc.scalar.copy
                eng(st[b][:, jj], ps[b])
            if jj == 1:
                for b in range(B):
                    nc.scalar.dma_start(out_d[b, ftp // 2], st[b])

    body = body_base if sched == "base" else body_ws

    with tile.TileContext(nc) as tc:
        with (
            tc.tile_pool(name="xsb", bufs=2) as xpool,
            tc.tile_pool(name="gsb", bufs=2) as gpool,
            tc.tile_pool(name="stage", bufs=3) as spool,
            tc.tile_pool(
                name="psum", bufs=8 if sched == "base" else 2, space="PSUM"
            ) as ppool,
        ):
            if iters > 1:
                with tc.For_i(0, iters, 1):
                    body(nc, tc, xpool, gpool, spool, ppool)
            else:
                body(nc, tc, xpool, gpool, spool, ppool)

    nc.compile()
    return nc


def _get_nc(iters=1, sched=None):
    sched = sched or SCHED
    key = ("nc", iters, sched)
    if key not in _cache:
        _cache[key] = _build_nc(iters, sched)
    return _cache[key]


# --------------------------------------------------------------------------
# host prep / assembly
# --------------------------------------------------------------------------
def _prep_arrays(x, dlnf, sched=None):
    """Host prep: G matrices + transposed/sharded device input arrays."""
    sched = sched or SCHED
    dt = np.float32 if sched == "base" else ml_dtypes.bfloat16
    x = np.asarray(x, dtype=np.float32)
    dlnf = np.asarray(dlnf, dtype=np.float32)
    G = _build_G(dlnf)                                     # (16, 1024, 1026)
    xt_n = x.reshape(B, MQ, 4, 128).transpose(3, 0, 2, 1)  # (128, B, 4, MQ)
    xt = np.zeros((128, B, 4, MQP), dt)
    xt[:, :, :, :MQ] = xt_n.astype(dt)
    xt = np.ascontiguousarray(xt)
    Ge = G[:, :, _KEEP]                                    # (16, 1024, 1024)
    g_all = Ge.reshape(D, KC, 128, FE).transpose(2, 1, 0, 3)  # (128,KC,D,FE)
    in_maps = [
        {
            "xt": xt,
            "g": np.ascontiguousarray(
                g_all[:, :, c * D_PER : (c + 1) * D_PER]
                .reshape(128, KC, D_PER * FE)
                .astype(dt)
            ),
        }
        for c in range(NCORES)
    ]
    return in_maps


def _assemble(results, sched=None):
    """per-core out2 -> (B, NW, D, NF) complex64."""
    sched = sched or SCHED
    full = np.zeros((B, NW, D, FW), dtype=np.float32)
    for c, r in enumerate(results):
        o = np.asarray(r["out"]).astype(np.float32)[..., :NW]
        o = o.transpose(0, 4, 1, 3, 2).reshape(B, NW, D_PER, FE)
        for dd in range(D_PER):
            full[:, :, c * D_PER + dd, _KEEP] = o[:, :, dd]
    return full.view(np.complex64).reshape(B, NW, D, NF)


# --------------------------------------------------------------------------
# runner (jitted multi-core executable, cached across kernel() calls)
# --------------------------------------------------------------------------
def _make_sharded(nc, devices=None):
    import jax
    from jax.experimental.shard_map import shard_map
    from jax.sharding import Mesh, PartitionSpec

    from concourse import bass2jax as b2j
    import concourse.mybir as mybir

    b2j.install_neuronx_cc_hook()
    partition_name = nc.partition_id_tensor.name if nc.partition_id_tensor else None

    in_names, out_names, out_avals, zero_outs = [], [], [], []
    for alloc in nc.m.functions[0].allocations:
        if not isinstance(alloc, mybir.MemoryLocationSet):
            continue
        name = alloc.memorylocations[0].name
        if alloc.kind == "ExternalInput":
            if name != partition_name:
                in_names.append(name)
        elif alloc.kind == "ExternalOutput":
            out_names.append(name)
            shape = tuple(alloc.tensor_shape)
            dtype = mybir.dt.np(alloc.dtype)
            out_avals.append(jax.core.ShapedArray(shape, dtype))
            zero_outs.append(np.zeros(shape, dtype))
    all_names = in_names + out_names
    if partition_name is not None:
        all_names = all_names + [partition_name]

    def _body(*args):
        operands = list(args)
        if partition_name is not None:
            operands.append(b2j.partition_id_tensor())
        outs = b2j._bass_exec_p.bind(
            *operands,
            out_avals=tuple(out_avals),
            in_names=tuple(all_names),
            out_names=tuple(out_names),
            lowering_input_output_aliases=(),
            sim_require_finite=True,
            sim_require_nnan=True,
            nc=nc,
        )
        return tuple(outs)

    if devices is None:
        devices = jax.devices()[:NCORES]
    mesh = Mesh(np.asarray(devices), ("core",))
    nin = len(in_names) + len(zero_outs)
    sharded = jax.jit(
        shard_map(
            _body,
            mesh=mesh,
            in_specs=(PartitionSpec("core"),) * nin,
            out_specs=(PartitionSpec("core"),) * len(out_names),
            check_rep=False,
        ),
        keep_unused=True,
    )
    return sharded, in_names, out_names, out_avals, zero_outs


def _get_runner(iters, sched=None):
    sched = sched or SCHED
    key = ("runner", iters, sched)
    if key in _cache:
        return _cache[key]

    import jax

    nc = _get_nc(iters, sched)
    sharded, in_names, out_names, out_avals, zero_outs = _make_sharded(nc)

    def call(in_maps):
        concat_in = [
            np.concatenate([in_maps[c][name] for c in range(NCORES)], axis=0)
            for name in in_names
        ] + [
            np.zeros((NCORES * z.shape[0], *z.shape[1:]), z.dtype)
            for z in zero_outs
        ]
        out_arrs = sharded(*concat_in)
        jax.block_until_ready(out_arrs)
        return [
            {
                name: np.asarray(out_arrs[i]).reshape(
                    NCORES, *out_avals[i].shape
                )[c]
                for i, name in enumerate(out_names)
            }
            for c in range(NCORES)
        ]

    _cache[key] = call
    return call


def _get_fft8_runner(dlnf, iters):
    """Heterogeneous per-core programs: 8 single-device executables."""
    key = ("fft8run", dlnf.tobytes(), iters)
    if key in _cache:
        return _cache[key]

    import jax

    ncs = _get_fft8_ncs(dlnf, iters)
    devices = jax.devices()[:NCORES]
    cores = []
    for c in range(NCORES):
        sharded, in_names, out_names, out_avals, zero_outs = _make_sharded(
            ncs[c], devices=[devices[c]]
        )
        cores.append((sharded, in_names, out_names, out_avals, zero_outs))

    def call(in_maps):
        outs = []
        for c in range(NCORES):
            sharded, in_names, out_names, out_avals, zero_outs = cores[c]
            args = [
                jax.device_put(in_maps[c][n], devices[c]) for n in in_names
            ] + [
                jax.device_put(np.zeros(z.shape, z.dtype), devices[c])
                for z in zero_outs
            ]
            outs.append(sharded(*args))
        jax.block_until_ready(outs)
        results = []
        for c in range(NCORES):
            _, _, out_names, out_avals, _ = cores[c]
            results.append(
                {
                    n: np.asarray(outs[c][i]).reshape(out_avals[i].shape)
                    for i, n in enumerate(out_names)
                }
            )
        return results

    _cache[key] = call
    return call


def kernel(x, dlnf, n_hann_splits=1, **_unused):
    iters = int(os.environ.get("KERNEL_ITERS", "1"))
    dlnf32 = np.asarray(dlnf, dtype=np.float32)
    if SCHED == "fft8":
        try:
            in_maps = _prep_fft8(x, dlnf32)
            call = _get_fft8_runner(dlnf32, iters)
            return _assemble_fft8(call(in_maps))
        except Exception:
            import traceback

            traceback.print_exc()
            # fall through to the dense-G path
    in_maps = _prep_arrays(x, dlnf, sched="bf16ws")
    try:
        call = _get_runner(iters, sched="bf16ws")
        results = call(in_maps)
    except Exception:
        # robust fallback: the reference implementation of the SPMD runner
        from concourse.bass_utils import run_bass_kernel_spmd

        nc = _get_nc(iters, sched="bf16ws")
        res = run_bass_kernel_spmd(nc, in_maps, core_ids=list(range(NCORES)))
        results = res.results
    return _assemble(results, sched="bf16ws")


# --------------------------------------------------------------------------
# benchmarking: jit once, time repeated executions (no retrace/relower)
# --------------------------------------------------------------------------
def prepare_bench(x, dlnf, iters, sched=None):
    """Returns run() -> wall seconds for one execution of the iters-body NEFF."""
    import time

    import jax

    sched = sched or SCHED
    if sched == "fft8":
        dlnf32 = np.asarray(dlnf, dtype=np.float32)
        in_maps = _prep_fft8(x, dlnf32)
        ncs = _get_fft8_ncs(dlnf32, iters)
        devices = jax.devices()[:NCORES]
        cores = []
        for c in range(NCORES):
            sharded, in_names, out_names, out_avals, zero_outs = _make_sharded(
                ncs[c], devices=[devices[c]]
            )
            args = [
                jax.device_put(in_maps[c][n], devices[c]) for n in in_names
            ] + [
                jax.device_put(np.zeros(z.shape, z.dtype), devices[c])
                for z in zero_outs
            ]
            cores.append((sharded, args))
        outs = [s(*a) for s, a in cores]
        jax.block_until_ready(outs)

        def run():
            t0 = time.perf_counter()
            o = [s(*a) for s, a in cores]
            jax.block_until_ready(o)
            return time.perf_counter() - t0

        return run
    in_maps = _prep_arrays(x, dlnf, sched)
    nc = _get_nc(iters, sched)
    sharded, in_names, out_names, out_avals, zero_outs = _make_sharded(nc)
    concat_in = [
        np.concatenate([in_maps[c][name] for c in range(NCORES)], axis=0)
        for name in in_names
    ] + [np.zeros((NCORES * z.shape[0], *z.shape[1:]), z.dtype) for z in zero_outs]
    concat_in = [jax.device_put(a) for a in concat_in]

    out = sharded(*concat_in)
    jax.block_until_ready(out)

    def run():
        t0 = time.perf_counter()
        o = sharded(*concat_in)
        jax.block_until_ready(o)
        return time.perf_counter() - t0

    return run


if __name__ == "__main__":
    rng = np.random.default_rng(0)
    x = rng.standard_normal((B, N), dtype=np.float32)
    dlnf = rng.uniform(-0.5, 0.5, size=(D,)).astype(np.float32)
    out = kernel(x, dlnf, 1)
    print("out:", out.shape, out.dtype)


# revision 43
# speedup vs baseline: 1.3631x; 1.0742x over previous
"""DechirpSTFT Trainium2 kernel.

The PE on these cores is purely column-rate limited (~1.8-2.0 cols/ns;
instruction count and weight reloads are free), so the win comes from
reducing matmul column-passes.  Default schedule 'fft8' factorizes the
per-chirp operator (hann window -> lerp resample * jac -> rfft(1024))
instead of applying it as one dense 1024x1026 matrix:

  R (PE) : r = L_d @ xw      banded resample blocks; i(tau) is a monotone
           bijection so the band sum is ~15 chunk-passes/chirp vs 64 dense
  A (DVE): level-1 radix butterflies e/o = r_j +- r_{j+4} (double-width
           [128,2,512] ops) + the even-residue combines — 9 ops/slice.
           DVE/ACT measured at only ~123 elem/ns, so minimizing DVE
           element volume matters ~1:1 against wall clock.
  B (PE) : X[8q+s] = per-residue folded DFT-128.  s=1,3 contract the o
           butterflies DIRECTLY (4 chunks, twiddle combines folded into
           the weights) — trades +18us of cheap PE columns for -25us of
           expensive DVE ops.  Col-packed 128-real out tiles; host
           conj-mirrors fill s=5,6,7

Column-passes drop 262144 -> ~119K per core.  All-bf16 intermediates:
rel err 4.4e-3 vs the 2e-2 gate (fp8 e4m3 measured 2.7-3.7e-2, fails).
The band structure depends on dlnf, so per-core programs are compiled
per call (cached by dlnf bytes) and run as 8 single-device executables.

Scheduling details that mattered (all measured on HW):
  * DVE may read ONE PSUM operand: level-1 butterflies read r0-r3 from
    PSUM directly; r4-r7 are copied to SBUF by ACT.  Chains issue
    pairwise (r0,r4,r1,r5,..) so banks free early.
  * Engine queues are in-order: A(s+1) is emitted BEFORE B(s)'s
    evictions, and R(s+2) before B(s), so neither PE nor DVE ever waits
    a full stage.  X evictions split 3/5 between DVE and ACT.
  * fp8 DoubleRow, multi-bank matmul outputs (>512 cols), and DMA from
    PSUM are all rejected by HW/walrus.

Fallback schedule 'bf16ws' (dense G, bf16, weight-stationary, ~1.25x
slower) runs if anything in the fft8 path throws; 'base' is the original
fp32r variant, selectable via KSCHED for A/B.

Sharding: D=16 chirp rates, 2 per core across 8 cores (x replicated).
No cross-core communication.

Timing: no NTFF profiling is reachable through this axon client, so
test.py measures HW time as (wall(For_i(T)) - wall(For_i(1))) / (T-1)
with the jitted executable and device-resident inputs held across calls.
"""

import os
import sys

sys.path.insert(0, "/opt/trn_rl_repo")

import numpy as np
import ml_dtypes

# ---- problem constants (hardcoded; kernel.py must be self-contained) ----
B = 4
N = 262144
K = 1024
HOP = 512
NW = (N - K) // HOP + 1          # 511
KTAU = 1024
NF = 513                         # rfft bins
FW = 2 * NF                      # 1026 interleaved re/im
FE = 1024                        # live f2 cols per chirp (im0/imNyq dropped)
NWP = 512                        # padded window count
MQP = 513                        # padded m-quads so window 511 stays in-bounds
D = 16
NCORES = 8
D_PER = D // NCORES              # 2
KC = 8                           # contraction chunks of 128
MQ = N // 512                    # 512
NFT = 16                         # f2e weight tiles of 128 per core
EPS = 1e-8

SCHED = os.environ.get("KSCHED", "fft8")

_cache = {}


# --------------------------------------------------------------------------
# host-side G construction
# --------------------------------------------------------------------------
def _build_tables_np(dlnf):
    """Reference's per-chirp tables in numpy float32 (fallback path)."""
    dlnf = dlnf.astype(np.float32)
    beta = (2.0 * dlnf).astype(np.float32)
    small = np.abs(beta) < EPS
    beta_safe = np.where(small, np.float32(EPS), beta).astype(np.float32)
    e2b = np.exp(2.0 * beta_safe).astype(np.float32)

    tau = (2.0 * np.arange(KTAU, dtype=np.float32) / KTAU - 1.0).astype(np.float32)
    t_source = np.log(
        1.0 + (tau[None, :] + 1.0) / 2.0 * (e2b[:, None] - 1.0)
    ).astype(np.float32)
    t_source = (t_source / beta_safe[:, None] - 1.0).astype(np.float32)
    t_source = np.where(small[:, None], tau[None, :], t_source)

    tau_mid = np.float32(2.0 * (KTAU // 2) / KTAU - 1.0)
    t_mid = (
        np.log(1.0 + (tau_mid + 1.0) / 2.0 * (e2b - 1.0)) / beta_safe - 1.0
    ).astype(np.float32)
    t_mid = np.where(small, tau_mid, t_mid)

    jac = np.exp(-beta_safe[:, None] * (t_source - t_mid[:, None])).astype(np.float32)
    jac = np.where(small[:, None], np.float32(1.0), jac)

    idx = (np.float32(K / 2.0) * (t_source + 1.0)).astype(np.float32)
    idx_lo = np.clip(idx.astype(np.int32), 0, K - 2)
    frac = (idx - idx_lo.astype(np.float32)).astype(np.float32)
    return idx_lo, frac, jac


def _build_tables(dlnf):
    """Per-chirp tables, computed with jax on the CPU backend so the f32
    transcendentals (log/exp) match the reference bit-for-bit."""
    try:
        import jax
        import jax.numpy as jnp

        cpu = jax.devices("cpu")[0]
    except Exception:
        return _build_tables_np(dlnf)

    with jax.default_device(cpu):
        beta = 2.0 * jnp.asarray(dlnf, dtype=jnp.float32)
        small = jnp.abs(beta) < EPS
        beta_safe = jnp.where(small, EPS, beta)
        e2b = jnp.exp(2.0 * beta_safe)

        tau = 2.0 * jnp.arange(KTAU, dtype=jnp.float32) / KTAU - 1.0
        t_source = (
            jnp.log(1.0 + (tau[None, :] + 1.0) / 2.0 * (e2b[:, None] - 1.0))
            / beta_safe[:, None]
            - 1.0
        )
        t_source = jnp.where(small[:, None], tau[None, :], t_source)

        tau_mid = 2.0 * (KTAU // 2) / KTAU - 1.0
        t_mid = (
            jnp.log(1.0 + (tau_mid + 1.0) / 2.0 * (e2b - 1.0)) / beta_safe - 1.0
        )
        t_mid = jnp.where(small, tau_mid, t_mid)

        jac = jnp.exp(-beta_safe[:, None] * (t_source - t_mid[:, None]))
        jac = jnp.where(small[:, None], 1.0, jac)

        idx = (K / 2.0) * (t_source + 1.0)
        idx_lo = jnp.clip(idx.astype(jnp.int32), 0, K - 2)
        frac = idx - idx_lo.astype(jnp.float32)
    return np.asarray(idx_lo), np.asarray(frac), np.asarray(jac)


def _build_G(dlnf):
    """G[d, k, f2] f32: fused hann * lerp-resample * jac * rfft operator."""
    nd = dlnf.shape[0]
    idx_lo, frac, jac = _build_tables(dlnf)
    t = np.arange(KTAU, dtype=np.float64)
    f = np.arange(NF, dtype=np.float64)
    ang = 2.0 * np.pi * np.outer(t, f) / KTAU
    Wre = np.cos(ang)
    Wim = -np.sin(ang)
    n = np.arange(K, dtype=np.float32)
    hann = (0.5 * (1.0 - np.cos(2.0 * np.pi * n / K))).astype(np.float32)

    G = np.zeros((nd, K, FW), dtype=np.float64)
    for d in range(nd):
        c_lo = (jac[d] * (1.0 - frac[d])).astype(np.float64)
        c_hi = (jac[d] * frac[d]).astype(np.float64)
        Gre = np.zeros((K, NF))
        Gim = np.zeros((K, NF))
        np.add.at(Gre, idx_lo[d], c_lo[:, None] * Wre)
        np.add.at(Gim, idx_lo[d], c_lo[:, None] * Wim)
        np.add.at(Gre, idx_lo[d] + 1, c_hi[:, None] * Wre)
        np.add.at(Gim, idx_lo[d] + 1, c_hi[:, None] * Wim)
        G[d, :, 0::2] = Gre
        G[d, :, 1::2] = Gim
    G *= hann[None, :, None].astype(np.float64)
    return G.astype(np.float32)


# live G columns: drop im(bin0) (col 1) and im(Nyquist) (col 1025)
_KEEP = np.concatenate(([0], np.arange(2, 1025)))


# --------------------------------------------------------------------------
# fft8: factorized pipeline  r = L_d @ xw  ->  radix-8 combine (DVE)
#   -> per-residue folded DFT-128 (PE).  ~119K PE column-passes vs 262K
# for the dense-G matmul.  Math validated in mock_fft8.py (bf16 rel err
# 3.2e-3 vs the 2e-2 gate).
# --------------------------------------------------------------------------
NST = 8          # output stiles per slice: [s0, s4, s1t0, s1t1, s2t0.., s3t1]
NBW = 22         # stage-B weight tiles (s1/s3 contract o0..o3 directly)
SQ2H = float(np.sqrt(2.0) / 2.0)


def _build_L_dense(dlnf):
    """L[d, tau, n] f32 resample operator with hann folded."""
    idx_lo, frac, jac = _build_tables(dlnf)
    n = np.arange(K, dtype=np.float32)
    hann = (0.5 * (1.0 - np.cos(2.0 * np.pi * n / K))).astype(np.float32)
    nd = dlnf.shape[0]
    L = np.zeros((nd, K, K), dtype=np.float32)
    rows = np.arange(K)
    for d in range(nd):
        c_lo = jac[d] * (1.0 - frac[d])
        c_hi = jac[d] * frac[d]
        L[d, rows, idx_lo[d]] += c_lo * hann[idx_lo[d]]
        L[d, rows, idx_lo[d] + 1] += c_hi * hann[idx_lo[d] + 1]
    return L, idx_lo


def _fft8_bweights():
    """bw [128, 22, 128] f32: stage-B weights W_s'[t,q]=w1024^{ts} W128^{tq},
    columns packed (q, re/im).  s=1,3 contract DIRECTLY over the level-1
    odd butterflies o0..o3 (4 chunks): the u1/u3 twiddle combines
      u1re = o0 + k(o1-o3);  u1im = -k*o1 - o2 - k*o3
      u3re = o0 - k(o1-o3);  u3im = -k*o1 + o2 - k*o3   (k = sqrt(2)/2)
    are folded into per-chunk weights, removing 6 DVE ops per slice."""
    if "bw" in _cache:
        return _cache["bw"]
    t = np.arange(128)[:, None]
    tiles = []
    for s in (0, 4):
        qs = np.arange(65) if s == 0 else np.arange(64)
        W = np.exp(-2j * np.pi * t * (8 * qs[None, :] + s) / 1024.0)
        cols = []
        for qi, q in enumerate(qs):
            if s == 0 and q in (0, 64):
                cols.append(W[:, qi].real)
            else:
                cols.append(W[:, qi].real)
                cols.append(W[:, qi].imag)
        tiles.append(np.stack(cols, 1))
    k = np.sqrt(2.0) / 2.0
    for s in (1, 2, 3):
        for ot in range(2):
            qs = np.arange(64) + 64 * ot
            W = np.exp(-2j * np.pi * t * (8 * qs[None, :] + s) / 1024.0)
            cre, cim = [], []
            for qi in range(64):
                cre.append(W[:, qi].real)
                cre.append(W[:, qi].imag)
                cim.append(-W[:, qi].imag)
                cim.append(W[:, qi].real)
            Wre = np.stack(cre, 1)   # multiplies u_re
            Wim = np.stack(cim, 1)   # multiplies true u_im
            if s == 2:
                tiles.append(Wre)
                tiles.append(Wim)
            elif s == 1:
                tiles.append(Wre)                    # o0
                tiles.append(k * (Wre - Wim))        # o1
                tiles.append(-Wim)                   # o2
                tiles.append(-k * (Wre + Wim))       # o3
            else:  # s == 3
                tiles.append(Wre)                    # o0
                tiles.append(-k * (Wre + Wim))       # o1
                tiles.append(Wim)                    # o2
                tiles.append(k * (Wre - Wim))        # o3
    bw = np.stack(tiles, 1).astype(np.float32)  # [128, 22, 128]
    _cache["bw"] = bw
    return bw


# stage-B matmul list: (bw_idx, operand_name, start, stop) per output stile
# tile order matches _fft8_bweights: [s0, s4, s1t0(4), s1t1(4), s2t0(2),
# s2t1(2), s3t0(4), s3t1(4)] -> indices 0,1, 2-5, 6-9, 10-11, 12-13, 14-21
_BMMS = [
    [(0, "u0", True, True)],
    [(1, "u4", True, True)],
    [(2, "o0", True, False), (3, "o1", False, False),
     (4, "o2", False, False), (5, "o3", False, True)],
    [(6, "o0", True, False), (7, "o1", False, False),
     (8, "o2", False, False), (9, "o3", False, True)],
    [(10, "u2re", True, False), (11, "u2im", False, True)],
    [(12, "u2re", True, False), (13, "u2im", False, True)],
    [(14, "o0", True, False), (15, "o1", False, False),
     (16, "o2", False, False), (17, "o3", False, True)],
    [(18, "o0", True, False), (19, "o1", False, False),
     (20, "o2", False, False), (21, "o3", False, True)],
]
_ST_S = [(0, 0), (4, 0), (1, 0), (1, 1), (2, 0), (2, 1), (3, 0), (3, 1)]


def _fft8_maps():
    """Static per-stile unpack maps: row -> (k, comp, sign)."""
    if "maps" in _cache:
        return _cache["maps"]
    maps = []
    for st in range(NST):
        s, ot = _ST_S[st]
        kk = np.zeros(128, np.int64)
        cc = np.zeros(128, np.int64)
        sg = np.zeros(128, np.float32)
        if s == 0:
            row = 0
            for q in range(65):
                if q in (0, 64):
                    kk[row], cc[row], sg[row] = 8 * q, 0, 1.0
                    row += 1
                else:
                    kk[row], cc[row], sg[row] = 8 * q, 0, 1.0
                    kk[row + 1], cc[row + 1], sg[row + 1] = 8 * q, 1, 1.0
                    row += 2
            assert row == 128
        elif s == 4:
            for qi in range(64):
                kk[2 * qi], cc[2 * qi], sg[2 * qi] = 8 * qi + 4, 0, 1.0
                kk[2 * qi + 1], cc[2 * qi + 1], sg[2 * qi + 1] = 8 * qi + 4, 1, 1.0
        else:
            for qi in range(64):
                q = qi + 64 * ot
                k = 8 * q + s
                if k <= 512:
                    kk[2 * qi], cc[2 * qi], sg[2 * qi] = k, 0, 1.0
                    kk[2 * qi + 1], cc[2 * qi + 1], sg[2 * qi + 1] = k, 1, 1.0
                else:
                    km = 1024 - k
                    assert 0 <= km <= 512
                    kk[2 * qi], cc[2 * qi], sg[2 * qi] = km, 0, 1.0
                    kk[2 * qi + 1], cc[2 * qi + 1], sg[2 * qi + 1] = km, 1, -1.0
        maps.append((kk, cc, sg))
    _cache["maps"] = maps
    return maps


def _fft8_structure(dlnf):
    """Per-core chain structure + L-weight blocks (lhsT [n,tau] layout)."""
    L, idx_lo = _build_L_dense(dlnf)
    cores = []
    for c in range(NCORES):
        chains = []  # (dl, I, [J...], blk_base)
        blocks = []
        for dl in range(D_PER):
            d = D_PER * c + dl
            for I in range(8):
                seg = idx_lo[d, 128 * I : 128 * (I + 1)]
                lo = int(seg.min()) // 128
                hi = int(seg.max() + 1) // 128
                js = list(range(lo, hi + 1))
                chains.append((dl, I, js, len(blocks)))
                for J in js:
                    blk = L[d, 128 * I : 128 * (I + 1), 128 * J : 128 * (J + 1)]
                    blocks.append(np.ascontiguousarray(blk.T))  # [n, tau]
        lw = np.stack(blocks, 1).astype(ml_dtypes.bfloat16)  # [128, nblk, 128]
        cores.append({"chains": chains, "lw": lw, "nblk": len(blocks)})
    return cores


def _build_nc_fft8(struct, iters=1):
    import concourse.bacc as bacc
    import concourse.mybir as mybir
    from concourse import tile

    f32 = mybir.dt.float32
    bf16 = mybir.dt.bfloat16
    nblk = struct["nblk"]
    chains = struct["chains"]

    nc = bacc.Bacc("TRN2", target_bir_lowering=False, debug=False)
    xt_d = nc.dram_tensor("xt", [128, B, 4, MQP], bf16, kind="ExternalInput")
    lw_d = nc.dram_tensor("lw", [128, nblk, 128], bf16, kind="ExternalInput")
    bw_d = nc.dram_tensor("bw", [128, NBW, 128], bf16, kind="ExternalInput")
    # out[slice = 4*dl + b, stile, p, w]
    out_d = nc.dram_tensor("out", [8, NST, 128, NWP], bf16, kind="ExternalOutput")

    A = mybir.AluOpType

    def body(nc, tc, pools):
        (xpool, lwpool, rsb, upool, tpool, xst, rps, rss, bps) = pools
        x_sb = xpool.tile([128, B, 4, MQP], bf16, name="x_sb")
        lw_sb = lwpool.tile([128, nblk, 128], bf16, name="lw_sb")
        bw_sb = lwpool.tile([128, NBW, 128], bf16, name="bw_sb")
        nc.sync.dma_start(lw_sb, lw_d[:])
        for b in range(B):
            nc.sync.dma_start(x_sb[:, b], xt_d[:, b])
        nc.sync.dma_start(bw_sb, bw_d[:])

        rtiles = [None, None]  # per pipeline parity: dict I -> sbuf tile
        utiles = [None, None]

        def emit_R(s):
            dl, b = s // 4, s % 4
            ch = {I: (js, base) for (cdl, I, js, base) in chains if cdl == dl}

            def chain(ps_half, I):
                js, base = ch[I]
                for ci, J in enumerate(js):
                    nc.tensor.matmul(
                        ps_half,
                        lw_sb[:, base + ci],
                        x_sb[:, b, J % 4, J // 4 : J // 4 + NWP],
                        start=(ci == 0),
                        stop=(ci == len(js) - 1),
                    )

            # 2-bank PSUM pairs coarsen the sync grain: one DVE butterfly op
            # and one ACT copy per PAIR instead of per tile.  Longs [r0|r1],
            # [r2|r3] stay in PSUM (the one PSUM operand DVE allows) on a
            # 2-deep rotation; shorts [r4|r5], [r6|r7] are copied to SBUF.
            L01 = rps.tile([128, 2, NWP], f32, name="L01", tag="rp")
            S45p = rss.tile([128, 2, NWP], f32, name="S45p", tag="rq")
            chain(L01[:, 0], 0)
            chain(S45p[:, 0], 4)
            chain(L01[:, 1], 1)
            chain(S45p[:, 1], 5)
            S45 = rsb.tile([128, 2, NWP], bf16, name="s45", tag="s45")
            nc.scalar.copy(S45, S45p)
            L23 = rps.tile([128, 2, NWP], f32, name="L23", tag="rp")
            S67p = rss.tile([128, 2, NWP], f32, name="S67p", tag="rq")
            # L23 chains first: S67p reuses S45p's banks (rss bufs=1), so its
            # first chain must trail the S45->SBUF copy; the two L23 chains
            # in between give that copy exactly the headroom it needs
            chain(L23[:, 0], 2)
            chain(L23[:, 1], 3)
            chain(S67p[:, 0], 6)
            chain(S67p[:, 1], 7)
            S67 = rsb.tile([128, 2, NWP], bf16, name="s67", tag="s67")
            nc.scalar.copy(S67, S67p)
            rtiles[s % 2] = (L01, L23, S45, S67)

        def emit_A(s):
            L01, L23, S45, S67 = rtiles[s % 2]
            tw = {
                nm: tpool.tile([128, 2, NWP], bf16, name=nm, tag=nm)
                for nm in ("e01", "e23", "t12")
            }
            # o-tiles are read by stage B next slice -> double-buffered pool
            o01 = upool.tile([128, 2, NWP], bf16, name="o01", tag="o01")
            o23 = upool.tile([128, 2, NWP], bf16, name="o23", tag="o23")
            u = {
                nm: upool.tile([128, NWP], bf16, name=nm, tag=nm)
                for nm in ("u0", "u4", "u2re", "u2im")
            }
            v = nc.vector
            # level-1 butterflies, double-width: e01=[e0|e1]=[r0+r4|r1+r5]..
            v.tensor_tensor(tw["e01"], L01, S45, op=A.add)
            v.tensor_tensor(o01, L01, S45, op=A.subtract)
            v.tensor_tensor(tw["e23"], L23, S67, op=A.add)
            v.tensor_tensor(o23, L23, S67, op=A.subtract)
            v.tensor_tensor(tw["t12"], tw["e01"], tw["e23"], op=A.add)  # [t1|t2]
            v.tensor_tensor(u["u0"], tw["t12"][:, 0], tw["t12"][:, 1], op=A.add)
            v.tensor_tensor(u["u4"], tw["t12"][:, 0], tw["t12"][:, 1], op=A.subtract)
            v.tensor_tensor(u["u2re"], tw["e01"][:, 0], tw["e23"][:, 0], op=A.subtract)
            v.tensor_tensor(u["u2im"], tw["e23"][:, 1], tw["e01"][:, 1], op=A.subtract)
            # s=1,3 twiddle combines are folded into bw: stage B contracts
            # o0..o3 directly (4 chunks), saving 6 DVE ops per slice
            u["o0"] = o01[:, 0]
            u["o1"] = o01[:, 1]
            u["o2"] = o23[:, 0]
            u["o3"] = o23[:, 1]
            utiles[s % 2] = u

        def emit_B(s):
            u = utiles[s % 2]
            for st in range(NST):
                ps = bps.tile([128, NWP], f32, name="xp", tag="xp")
                for (bwi, unm, sa, so) in _BMMS[st]:
                    nc.tensor.matmul(
                        ps, bw_sb[:, bwi], u[unm], start=sa, stop=so
                    )
                xs = xst.tile([128, NWP], bf16, name="xs", tag="xs")
                # Eviction engine split: DVE evictions queue BEHIND the next
                # slice's stage-A block, so they must be the LAST two stiles
                # (their 2-bank-pool slots aren't needed until the next
                # slice); early stiles evict on ACT, whose queue is timely.
                # (st<3 on DVE stalled B mid-slice; all-ACT overloads ACT.)
                if st >= 6:
                    nc.vector.tensor_copy(xs, ps)
                else:
                    nc.scalar.copy(xs, ps)
                nc.scalar.dma_start(out_d[s, st], xs)

        # Emission order shapes each engine's in-order queue:
        #   PE : R0 R1 [B0 R2] [B1 R3] ... — resample of slice s+2 overlaps
        #        stage-B of slice s, so PE never waits on the DVE.
        #   DVE: A0 A1 ev(B0) A2 ev(B1) ... — A(s+1) is queued BEFORE the
        #        evictions of B(s); evictions never delay the next A block.
        emit_R(0)
        emit_R(1)
        emit_A(0)
        for s in range(8):
            if s + 1 < 8:
                emit_A(s + 1)
            emit_B(s)
            if s + 2 < 8:
                emit_R(s + 2)

    with tile.TileContext(nc) as tc:
        with (
            tc.tile_pool(name="xsb", bufs=2) as xpool,
            tc.tile_pool(name="lwsb", bufs=2) as lwpool,
            tc.tile_pool(name="rsb", bufs=2) as rsb,
            tc.tile_pool(name="usb", bufs=2) as upool,
            tc.tile_pool(name="tsb", bufs=1) as tpool,
            tc.tile_pool(name="xst", bufs=6) as xst,
            tc.tile_pool(name="rps", bufs=2, space="PSUM") as rps,
            tc.tile_pool(name="rss", bufs=1, space="PSUM") as rss,
            tc.tile_pool(name="bps", bufs=2, space="PSUM") as bps,
        ):
            pools = (xpool, lwpool, rsb, upool, tpool, xst, rps, rss, bps)
            if iters > 1:
                with tc.For_i(0, iters, 1):
                    body(nc, tc, pools)
            else:
                body(nc, tc, pools)

    nc.compile()
    return nc


def _get_fft8_ncs(dlnf, iters):
    key = ("fft8", dlnf.tobytes(), iters)
    if key not in _cache:
        if ("fft8s", dlnf.tobytes()) not in _cache:
            _cache[("fft8s", dlnf.tobytes())] = _fft8_structure(dlnf)
        structs = _cache[("fft8s", dlnf.tobytes())]
        _cache[key] = [_build_nc_fft8(s, iters) for s in structs]
    return _cache[key]


def _prep_fft8(x, dlnf):
    x = np.asarray(x, dtype=np.float32)
    dlnf = np.asarray(dlnf, dtype=np.float32)
    skey = ("fft8s", dlnf.tobytes())
    if skey not in _cache:
        _cache[skey] = _fft8_structure(dlnf)
    structs = _cache[skey]
    xt_n = x.reshape(B, MQ, 4, 128).transpose(3, 0, 2, 1)
    xt = np.zeros((128, B, 4, MQP), ml_dtypes.bfloat16)
    xt[:, :, :, :MQ] = xt_n.astype(ml_dtypes.bfloat16)
    xt = np.ascontiguousarray(xt)
    bw = _fft8_bweights().astype(ml_dtypes.bfloat16)
    return [
        {"xt": xt, "lw": np.ascontiguousarray(structs[c]["lw"]), "bw": bw}
        for c in range(NCORES)
    ]


def _assemble_fft8(results):
    maps = _fft8_maps()
    full = np.zeros((B, NW, D, NF, 2), dtype=np.float32)
    for c, r in enumerate(results):
        o = np.asarray(r["out"]).astype(np.float32)  # [8, 8, 128, NWP]
        for dl in range(D_PER):
            d = D_PER * c + dl
            for b in range(B):
                sl = o[4 * dl + b]  # [8, 128, NWP]
                for st in range(NST):
                    kk, cc, sg = maps[st]
                    full[b, :, d, kk, cc] = (
                        sl[st, :, :NW] * sg[:, None]
                    )
    return (
        full.reshape(B, NW, D, NF * 2)
        .view(np.complex64)
        .reshape(B, NW, D, NF)
    )


# --------------------------------------------------------------------------
# device program
# --------------------------------------------------------------------------
def _build_nc(iters=1, sched=None):
    import concourse.bacc as bacc
    import concourse.mybir as mybir
    from concourse import tile

    sched = sched or SCHED
    f32 = mybir.dt.float32
    bf16 = mybir.dt.bfloat16
    mm_dt = mybir.dt.float32r if sched == "base" else bf16

    nc = bacc.Bacc("TRN2", target_bir_lowering=False, debug=False)

    # xt[p, b, r, mq] = x[b, 128*(4*mq + r) + p]  (mq innermost: every
    # matmul moving slice is contiguous)
    xt_d = nc.dram_tensor("xt", [128, B, 4, MQP], mm_dt, kind="ExternalInput")
    # g[p, kc, 1024*d + fe] = G_d[128*kc + p, keep[fe]]
    g_d = nc.dram_tensor("g", [128, KC, D_PER * FE], mm_dt, kind="ExternalInput")
    if sched == "base":
        out_d = nc.dram_tensor(
            "out", [B, NFT // 4, 128, 4, NWP], f32, kind="ExternalOutput"
        )
    else:
        # out[b, fg, p, j, w]: f2e tile ft = 2*fg + j, psum partition p
        out_d = nc.dram_tensor(
            "out", [B, NFT // 2, 128, 2, NWP], bf16, kind="ExternalOutput"
        )

    def body_base(nc, tc, xpool, gpool, spool, ppool):
        x_sb = xpool.tile([128, B, 4, MQP], mm_dt, name="x_sb")
        g_sb = gpool.tile([128, KC, D_PER * FE], mm_dt, name="g_sb")
        for b in range(B):
            nc.sync.dma_start(x_sb[:, b], xt_d[:, b])
        for kc in range(KC):
            nc.sync.dma_start(g_sb[:, kc], g_d[:, kc])

        for b in range(B):
            for ftp in range(NFT // 2):
                st = spool.tile([128, 2, NWP], f32, name="st")
                for jj in range(2):
                    ft = 2 * ftp + jj
                    ps = ppool.tile([128, NWP], f32, name="ps", tag="ps")
                    for h in range(2):
                        for kc in range(KC):
                            q, r = divmod(kc, 4)
                            nc.tensor.matmul(
                                ps[:, h * 256 : (h + 1) * 256],
                                g_sb[:, kc, 128 * ft : 128 * (ft + 1)],
                                x_sb[:, b, r, q + h * 256 : q + h * 256 + 256],
                                start=(kc == 0 and h == 0),
                                stop=(kc == KC - 1 and h == 1),
                            )
                    eng = nc.vector.tensor_copy if ft % 2 == 0 else nc.scalar.copy
                    eng(st[:, jj], ps)
                nc.scalar.dma_start(
                    out_d[b, ftp // 2, :, 2 * (ftp % 2) : 2 * (ftp % 2) + 2], st
                )

    def body_ws(nc, tc, xpool, gpool, spool, ppool):
        x_sb = xpool.tile([128, B, 4, MQP], mm_dt, name="x_sb")
        g_sb = gpool.tile([128, KC, D_PER * FE], mm_dt, name="g_sb")
        nc.sync.dma_start(g_sb[:, 0], g_d[:, 0])
        for b in range(B):
            nc.sync.dma_start(x_sb[:, b], xt_d[:, b])
        for kc in range(1, KC):
            nc.sync.dma_start(g_sb[:, kc], g_d[:, kc])

        st = {}
        for ftp in range(NFT):
            ps = [
                ppool.tile([128, NWP], f32, name=f"ps{b}", tag=f"ps{b}")
                for b in range(B)
            ]
            for kc in range(KC):
                q, r = divmod(kc, 4)
                w_ap = g_sb[:, kc, 128 * ftp : 128 * (ftp + 1)]
                for b in range(B):
                    nc.tensor.matmul(
                        ps[b],
                        w_ap,
                        x_sb[:, b, r, q : q + NWP],
                        start=(kc == 0),
                        stop=(kc == KC - 1),
                    )
            jj = ftp % 2
            if jj == 0:
                for b in range(B):
                    st[b] = spool.tile([128, 2, NWP], bf16, name=f"st{b}")
            for b in range(B):
                eng = nc.vector.tensor_copy if b % 2 == 0 else nc.scalar.copy
                eng(st[b][:, jj], ps[b])
            if jj == 1:
                for b in range(B):
                    nc.scalar.dma_start(out_d[b, ftp // 2], st[b])

    body = body_base if sched == "base" else body_ws

    with tile.TileContext(nc) as tc:
        with (
            tc.tile_pool(name="xsb", bufs=2) as xpool,
            tc.tile_pool(name="gsb", bufs=2) as gpool,
            tc.tile_pool(name="stage", bufs=3) as spool,
            tc.tile_pool(
                name="psum", bufs=8 if sched == "base" else 2, space="PSUM"
            ) as ppool,
        ):
            if iters > 1:
                with tc.For_i(0, iters, 1):
                    body(nc, tc, xpool, gpool, spool, ppool)
            else:
                body(nc, tc, xpool, gpool, spool, ppool)

    nc.compile()
    return nc


def _get_nc(iters=1, sched=None):
    sched = sched or SCHED
    key = ("nc", iters, sched)
    if key not in _cache:
        _cache[key] = _build_nc(iters, sched)
    return _cache[key]


# --------------------------------------------------------------------------
# host prep / assembly
# --------------------------------------------------------------------------
def _prep_arrays(x, dlnf, sched=None):
    """Host prep: G matrices + transposed/sharded device input arrays."""
    sched = sched or SCHED
    dt = np.float32 if sched == "base" else ml_dtypes.bfloat16
    x = np.asarray(x, dtype=np.float32)
    dlnf = np.asarray(dlnf, dtype=np.float32)
    G = _build_G(dlnf)                                     # (16, 1024, 1026)
    xt_n = x.reshape(B, MQ, 4, 128).transpose(3, 0, 2, 1)  # (128, B, 4, MQ)
    xt = np.zeros((128, B, 4, MQP), dt)
    xt[:, :, :, :MQ] = xt_n.astype(dt)
    xt = np.ascontiguousarray(xt)
    Ge = G[:, :, _KEEP]                                    # (16, 1024, 1024)
    g_all = Ge.reshape(D, KC, 128, FE).transpose(2, 1, 0, 3)  # (128,KC,D,FE)
    in_maps = [
        {
            "xt": xt,
            "g": np.ascontiguousarray(
                g_all[:, :, c * D_PER : (c + 1) * D_PER]
                .reshape(128, KC, D_PER * FE)
                .astype(dt)
            ),
        }
        for c in range(NCORES)
    ]
    return in_maps


def _assemble(results, sched=None):
    """per-core out2 -> (B, NW, D, NF) complex64."""
    sched = sched or SCHED
    full = np.zeros((B, NW, D, FW), dtype=np.float32)
    for c, r in enumerate(results):
        o = np.asarray(r["out"]).astype(np.float32)[..., :NW]
        o = o.transpose(0, 4, 1, 3, 2).reshape(B, NW, D_PER, FE)
        for dd in range(D_PER):
            full[:, :, c * D_PER + dd, _KEEP] = o[:, :, dd]
    return full.view(np.complex64).reshape(B, NW, D, NF)


# --------------------------------------------------------------------------
# runner (jitted multi-core executable, cached across kernel() calls)
# --------------------------------------------------------------------------
def _make_sharded(nc, devices=None):
    import jax
    from jax.experimental.shard_map import shard_map
    from jax.sharding import Mesh, PartitionSpec

    from concourse import bass2jax as b2j
    import concourse.mybir as mybir

    b2j.install_neuronx_cc_hook()
    partition_name = nc.partition_id_tensor.name if nc.partition_id_tensor else None

    in_names, out_names, out_avals, zero_outs = [], [], [], []
    for alloc in nc.m.functions[0].allocations:
        if not isinstance(alloc, mybir.MemoryLocationSet):
            continue
        name = alloc.memorylocations[0].name
        if alloc.kind == "ExternalInput":
            if name != partition_name:
                in_names.append(name)
        elif alloc.kind == "ExternalOutput":
            out_names.append(name)
            shape = tuple(alloc.tensor_shape)
            dtype = mybir.dt.np(alloc.dtype)
            out_avals.append(jax.core.ShapedArray(shape, dtype))
            zero_outs.append(np.zeros(shape, dtype))
    all_names = in_names + out_names
    if partition_name is not None:
        all_names = all_names + [partition_name]

    def _body(*args):
        operands = list(args)
        if partition_name is not None:
            operands.append(b2j.partition_id_tensor())
        outs = b2j._bass_exec_p.bind(
            *operands,
            out_avals=tuple(out_avals),
            in_names=tuple(all_names),
            out_names=tuple(out_names),
            lowering_input_output_aliases=(),
            sim_require_finite=True,
            sim_require_nnan=True,
            nc=nc,
        )
        return tuple(outs)

    if devices is None:
        devices = jax.devices()[:NCORES]
    mesh = Mesh(np.asarray(devices), ("core",))
    nin = len(in_names) + len(zero_outs)
    sharded = jax.jit(
        shard_map(
            _body,
            mesh=mesh,
            in_specs=(PartitionSpec("core"),) * nin,
            out_specs=(PartitionSpec("core"),) * len(out_names),
            check_rep=False,
        ),
        keep_unused=True,
    )
    return sharded, in_names, out_names, out_avals, zero_outs


def _get_runner(iters, sched=None):
    sched = sched or SCHED
    key = ("runner", iters, sched)
    if key in _cache:
        return _cache[key]

    import jax

    nc = _get_nc(iters, sched)
    sharded, in_names, out_names, out_avals, zero_outs = _make_sharded(nc)

    def call(in_maps):
        concat_in = [
            np.concatenate([in_maps[c][name] for c in range(NCORES)], axis=0)
            for name in in_names
        ] + [
            np.zeros((NCORES * z.shape[0], *z.shape[1:]), z.dtype)
            for z in zero_outs
        ]
        out_arrs = sharded(*concat_in)
        jax.block_until_ready(out_arrs)
        return [
            {
                name: np.asarray(out_arrs[i]).reshape(
                    NCORES, *out_avals[i].shape
                )[c]
                for i, name in enumerate(out_names)
            }
            for c in range(NCORES)
        ]

    _cache[key] = call
    return call


def _get_fft8_runner(dlnf, iters):
    """Heterogeneous per-core programs: 8 single-device executables."""
    key = ("fft8run", dlnf.tobytes(), iters)
    if key in _cache:
        return _cache[key]

    import jax

    ncs = _get_fft8_ncs(dlnf, iters)
    devices = jax.devices()[:NCORES]
    cores = []
    for c in range(NCORES):
        sharded, in_names, out_names, out_avals, zero_outs = _make_sharded(
            ncs[c], devices=[devices[c]]
        )
        cores.append((sharded, in_names, out_names, out_avals, zero_outs))

    def call(in_maps):
        outs = []
        for c in range(NCORES):
            sharded, in_names, out_names, out_avals, zero_outs = cores[c]
            args = [
                jax.device_put(in_maps[c][n], devices[c]) for n in in_names
            ] + [
                jax.device_put(np.zeros(z.shape, z.dtype), devices[c])
                for z in zero_outs
            ]
            outs.append(sharded(*args))
        jax.block_until_ready(outs)
        results = []
        for c in range(NCORES):
            _, _, out_names, out_avals, _ = cores[c]
            results.append(
                {
                    n: np.asarray(outs[c][i]).reshape(out_avals[i].shape)
                    for i, n in enumerate(out_names)
                }
            )
        return results

    _cache[key] = call
    return call


def kernel(x, dlnf, n_hann_splits=1, **_unused):
    iters = int(os.environ.get("KERNEL_ITERS", "1"))
    dlnf32 = np.asarray(dlnf, dtype=np.float32)
    if SCHED == "fft8":
        try:
            in_maps = _prep_fft8(x, dlnf32)
            call = _get_fft8_runner(dlnf32, iters)
            return _assemble_fft8(call(in_maps))
        except Exception:
            import traceback

            traceback.print_exc()
            # fall through to the dense-G path
    in_maps = _prep_arrays(x, dlnf, sched="bf16ws")
    try:
        call = _get_runner(iters, sched="bf16ws")
        results = call(in_maps)
    except Exception:
        # robust fallback: the reference implementation of the SPMD runner
        from concourse.bass_utils import run_bass_kernel_spmd

        nc = _get_nc(iters, sched="bf16ws")
        res = run_bass_kernel_spmd(nc, in_maps, core_ids=list(range(NCORES)))
        results = res.results
    return _assemble(results, sched="bf16ws")


# --------------------------------------------------------------------------
# benchmarking: jit once, time repeated executions (no retrace/relower)
# --------------------------------------------------------------------------
def prepare_bench(x, dlnf, iters, sched=None):
    """Returns run() -> wall seconds for one execution of the iters-body NEFF."""
    import time

    import jax

    sched = sched or SCHED
    if sched == "fft8":
        dlnf32 = np.asarray(dlnf, dtype=np.float32)
        in_maps = _prep_fft8(x, dlnf32)
        ncs = _get_fft8_ncs(dlnf32, iters)
        devices = jax.devices()[:NCORES]
        cores = []
        for c in range(NCORES):
            sharded, in_names, out_names, out_avals, zero_outs = _make_sharded(
                ncs[c], devices=[devices[c]]
            )
            args = [
                jax.device_put(in_maps[c][n], devices[c]) for n in in_names
            ] + [
                jax.device_put(np.zeros(z.shape, z.dtype), devices[c])
                for z in zero_outs
            ]
            cores.append((sharded, args))
        outs = [s(*a) for s, a in cores]
        jax.block_until_ready(outs)

        def run():
            t0 = time.perf_counter()
            o = [s(*a) for s, a in cores]
            jax.block_until_ready(o)
            return time.perf_counter() - t0

        return run
    in_maps = _prep_arrays(x, dlnf, sched)
    nc = _get_nc(iters, sched)
    sharded, in_names, out_names, out_avals, zero_outs = _make_sharded(nc)
    concat_in = [
        np.concatenate([in_maps[c][name] for c in range(NCORES)], axis=0)
        for name in in_names
    ] + [np.zeros((NCORES * z.shape[0], *z.shape[1:]), z.dtype) for z in zero_outs]
    concat_in = [jax.device_put(a) for a in concat_in]

    out = sharded(*concat_in)
    jax.block_until_ready(out)

    def run():
        t0 = time.perf_counter()
        o = sharded(*concat_in)
        jax.block_until_ready(o)
        return time.perf_counter() - t0

    return run


if __name__ == "__main__":
    rng = np.random.default_rng(0)
    x = rng.standard_normal((B, N), dtype=np.float32)
    dlnf = rng.uniform(-0.5, 0.5, size=(D,)).astype(np.float32)
    out = kernel(x, dlnf, 1)
    print("out:", out.shape, out.dtype)


# revision 45
# speedup vs baseline: 1.3682x; 1.0038x over previous
"""DechirpSTFT Trainium2 kernel.

The PE on these cores is purely column-rate limited (~1.8-2.0 cols/ns;
instruction count and weight reloads are free), so the win comes from
reducing matmul column-passes.  Default schedule 'fft8' factorizes the
per-chirp operator (hann window -> lerp resample * jac -> rfft(1024))
instead of applying it as one dense 1024x1026 matrix:

  R (PE) : r = L_d @ xw      banded resample blocks; i(tau) is a monotone
           bijection so the band sum is ~15 chunk-passes/chirp vs 64 dense
  A (DVE): level-1 radix butterflies e/o = r_j +- r_{j+4} (double-width
           [128,2,512] ops) + the even-residue combines — 9 ops/slice.
           DVE/ACT measured at only ~123 elem/ns, so minimizing DVE
           element volume matters ~1:1 against wall clock.
  B (PE) : X[8q+s] = per-residue folded DFT-128.  s=1,3 contract the o
           butterflies DIRECTLY (4 chunks, twiddle combines folded into
           the weights) — trades +18us of cheap PE columns for -25us of
           expensive DVE ops.  Col-packed 128-real out tiles; host
           conj-mirrors fill s=5,6,7

Column-passes drop 262144 -> ~119K per core.  All-bf16 intermediates:
rel err 4.4e-3 vs the 2e-2 gate (fp8 e4m3 measured 2.7-3.7e-2, fails).
The band structure depends on dlnf, so per-core programs are compiled
per call (cached by dlnf bytes) and run as 8 single-device executables.

Scheduling details that mattered (all measured on HW):
  * DVE may read ONE PSUM operand: level-1 butterflies read r0-r3 from
    PSUM directly; r4-r7 are copied to SBUF by ACT.  Chains issue
    pairwise (r0,r4,r1,r5,..) so banks free early.
  * Engine queues are in-order: A(s+1) is emitted BEFORE B(s)'s
    evictions, and R(s+2) before B(s), so neither PE nor DVE ever waits
    a full stage.  X evictions split 3/5 between DVE and ACT.
  * fp8 DoubleRow, multi-bank matmul outputs (>512 cols), and DMA from
    PSUM are all rejected by HW/walrus.

Fallback schedule 'bf16ws' (dense G, bf16, weight-stationary, ~1.25x
slower) runs if anything in the fft8 path throws; 'base' is the original
fp32r variant, selectable via KSCHED for A/B.

Sharding: D=16 chirp rates, 2 per core across 8 cores (x replicated).
No cross-core communication.

Timing: no NTFF profiling is reachable through this axon client, so
test.py measures HW time as (wall(For_i(T)) - wall(For_i(1))) / (T-1)
with the jitted executable and device-resident inputs held across calls.
"""

import os
import sys

sys.path.insert(0, "/opt/trn_rl_repo")

import numpy as np
import ml_dtypes

# ---- problem constants (hardcoded; kernel.py must be self-contained) ----
B = 4
N = 262144
K = 1024
HOP = 512
NW = (N - K) // HOP + 1          # 511
KTAU = 1024
NF = 513                         # rfft bins
FW = 2 * NF                      # 1026 interleaved re/im
FE = 1024                        # live f2 cols per chirp (im0/imNyq dropped)
NWP = 512                        # padded window count
MQP = 513                        # padded m-quads so window 511 stays in-bounds
D = 16
NCORES = 8
D_PER = D // NCORES              # 2
KC = 8                           # contraction chunks of 128
MQ = N // 512                    # 512
NFT = 16                         # f2e weight tiles of 128 per core
EPS = 1e-8

SCHED = os.environ.get("KSCHED", "fft8")

_cache = {}


# --------------------------------------------------------------------------
# host-side G construction
# --------------------------------------------------------------------------
def _build_tables_np(dlnf):
    """Reference's per-chirp tables in numpy float32 (fallback path)."""
    dlnf = dlnf.astype(np.float32)
    beta = (2.0 * dlnf).astype(np.float32)
    small = np.abs(beta) < EPS
    beta_safe = np.where(small, np.float32(EPS), beta).astype(np.float32)
    e2b = np.exp(2.0 * beta_safe).astype(np.float32)

    tau = (2.0 * np.arange(KTAU, dtype=np.float32) / KTAU - 1.0).astype(np.float32)
    t_source = np.log(
        1.0 + (tau[None, :] + 1.0) / 2.0 * (e2b[:, None] - 1.0)
    ).astype(np.float32)
    t_source = (t_source / beta_safe[:, None] - 1.0).astype(np.float32)
    t_source = np.where(small[:, None], tau[None, :], t_source)

    tau_mid = np.float32(2.0 * (KTAU // 2) / KTAU - 1.0)
    t_mid = (
        np.log(1.0 + (tau_mid + 1.0) / 2.0 * (e2b - 1.0)) / beta_safe - 1.0
    ).astype(np.float32)
    t_mid = np.where(small, tau_mid, t_mid)

    jac = np.exp(-beta_safe[:, None] * (t_source - t_mid[:, None])).astype(np.float32)
    jac = np.where(small[:, None], np.float32(1.0), jac)

    idx = (np.float32(K / 2.0) * (t_source + 1.0)).astype(np.float32)
    idx_lo = np.clip(idx.astype(np.int32), 0, K - 2)
    frac = (idx - idx_lo.astype(np.float32)).astype(np.float32)
    return idx_lo, frac, jac


def _build_tables(dlnf):
    """Per-chirp tables, computed with jax on the CPU backend so the f32
    transcendentals (log/exp) match the reference bit-for-bit."""
    try:
        import jax
        import jax.numpy as jnp

        cpu = jax.devices("cpu")[0]
    except Exception:
        return _build_tables_np(dlnf)

    with jax.default_device(cpu):
        beta = 2.0 * jnp.asarray(dlnf, dtype=jnp.float32)
        small = jnp.abs(beta) < EPS
        beta_safe = jnp.where(small, EPS, beta)
        e2b = jnp.exp(2.0 * beta_safe)

        tau = 2.0 * jnp.arange(KTAU, dtype=jnp.float32) / KTAU - 1.0
        t_source = (
            jnp.log(1.0 + (tau[None, :] + 1.0) / 2.0 * (e2b[:, None] - 1.0))
            / beta_safe[:, None]
            - 1.0
        )
        t_source = jnp.where(small[:, None], tau[None, :], t_source)

        tau_mid = 2.0 * (KTAU // 2) / KTAU - 1.0
        t_mid = (
            jnp.log(1.0 + (tau_mid + 1.0) / 2.0 * (e2b - 1.0)) / beta_safe - 1.0
        )
        t_mid = jnp.where(small, tau_mid, t_mid)

        jac = jnp.exp(-beta_safe[:, None] * (t_source - t_mid[:, None]))
        jac = jnp.where(small[:, None], 1.0, jac)

        idx = (K / 2.0) * (t_source + 1.0)
        idx_lo = jnp.clip(idx.astype(jnp.int32), 0, K - 2)
        frac = idx - idx_lo.astype(jnp.float32)
    return np.asarray(idx_lo), np.asarray(frac), np.asarray(jac)


def _build_G(dlnf):
    """G[d, k, f2] f32: fused hann * lerp-resample * jac * rfft operator."""
    nd = dlnf.shape[0]
    idx_lo, frac, jac = _build_tables(dlnf)
    t = np.arange(KTAU, dtype=np.float64)
    f = np.arange(NF, dtype=np.float64)
    ang = 2.0 * np.pi * np.outer(t, f) / KTAU
    Wre = np.cos(ang)
    Wim = -np.sin(ang)
    n = np.arange(K, dtype=np.float32)
    hann = (0.5 * (1.0 - np.cos(2.0 * np.pi * n / K))).astype(np.float32)

    G = np.zeros((nd, K, FW), dtype=np.float64)
    for d in range(nd):
        c_lo = (jac[d] * (1.0 - frac[d])).astype(np.float64)
        c_hi = (jac[d] * frac[d]).astype(np.float64)
        Gre = np.zeros((K, NF))
        Gim = np.zeros((K, NF))
        np.add.at(Gre, idx_lo[d], c_lo[:, None] * Wre)
        np.add.at(Gim, idx_lo[d], c_lo[:, None] * Wim)
        np.add.at(Gre, idx_lo[d] + 1, c_hi[:, None] * Wre)
        np.add.at(Gim, idx_lo[d] + 1, c_hi[:, None] * Wim)
        G[d, :, 0::2] = Gre
        G[d, :, 1::2] = Gim
    G *= hann[None, :, None].astype(np.float64)
    return G.astype(np.float32)


# live G columns: drop im(bin0) (col 1) and im(Nyquist) (col 1025)
_KEEP = np.concatenate(([0], np.arange(2, 1025)))


# --------------------------------------------------------------------------
# fft8: factorized pipeline  r = L_d @ xw  ->  radix-8 combine (DVE)
#   -> per-residue folded DFT-128 (PE).  ~119K PE column-passes vs 262K
# for the dense-G matmul.  Math validated in mock_fft8.py (bf16 rel err
# 3.2e-3 vs the 2e-2 gate).
# --------------------------------------------------------------------------
NST = 8          # output stiles per slice: [s0, s4, s1t0, s1t1, s2t0.., s3t1]
NBW = 22         # stage-B weight tiles (s1/s3 contract o0..o3 directly)
SQ2H = float(np.sqrt(2.0) / 2.0)


def _build_L_dense(dlnf):
    """L[d, tau, n] f32 resample operator with hann folded."""
    idx_lo, frac, jac = _build_tables(dlnf)
    n = np.arange(K, dtype=np.float32)
    hann = (0.5 * (1.0 - np.cos(2.0 * np.pi * n / K))).astype(np.float32)
    nd = dlnf.shape[0]
    L = np.zeros((nd, K, K), dtype=np.float32)
    rows = np.arange(K)
    for d in range(nd):
        c_lo = jac[d] * (1.0 - frac[d])
        c_hi = jac[d] * frac[d]
        L[d, rows, idx_lo[d]] += c_lo * hann[idx_lo[d]]
        L[d, rows, idx_lo[d] + 1] += c_hi * hann[idx_lo[d] + 1]
    return L, idx_lo


def _fft8_bweights():
    """bw [128, 22, 128] f32: stage-B weights W_s'[t,q]=w1024^{ts} W128^{tq},
    columns packed (q, re/im).  s=1,3 contract DIRECTLY over the level-1
    odd butterflies o0..o3 (4 chunks): the u1/u3 twiddle combines
      u1re = o0 + k(o1-o3);  u1im = -k*o1 - o2 - k*o3
      u3re = o0 - k(o1-o3);  u3im = -k*o1 + o2 - k*o3   (k = sqrt(2)/2)
    are folded into per-chunk weights, removing 6 DVE ops per slice."""
    if "bw" in _cache:
        return _cache["bw"]
    t = np.arange(128)[:, None]
    tiles = []
    for s in (0, 4):
        qs = np.arange(65) if s == 0 else np.arange(64)
        W = np.exp(-2j * np.pi * t * (8 * qs[None, :] + s) / 1024.0)
        cols = []
        for qi, q in enumerate(qs):
            if s == 0 and q in (0, 64):
                cols.append(W[:, qi].real)
            else:
                cols.append(W[:, qi].real)
                cols.append(W[:, qi].imag)
        tiles.append(np.stack(cols, 1))
    k = np.sqrt(2.0) / 2.0
    for s in (1, 2, 3):
        for ot in range(2):
            qs = np.arange(64) + 64 * ot
            W = np.exp(-2j * np.pi * t * (8 * qs[None, :] + s) / 1024.0)
            cre, cim = [], []
            for qi in range(64):
                cre.append(W[:, qi].real)
                cre.append(W[:, qi].imag)
                cim.append(-W[:, qi].imag)
                cim.append(W[:, qi].real)
            Wre = np.stack(cre, 1)   # multiplies u_re
            Wim = np.stack(cim, 1)   # multiplies true u_im
            if s == 2:
                tiles.append(Wre)
                tiles.append(Wim)
            elif s == 1:
                tiles.append(Wre)                    # o0
                tiles.append(k * (Wre - Wim))        # o1
                tiles.append(-Wim)                   # o2
                tiles.append(-k * (Wre + Wim))       # o3
            else:  # s == 3
                tiles.append(Wre)                    # o0
                tiles.append(-k * (Wre + Wim))       # o1
                tiles.append(Wim)                    # o2
                tiles.append(k * (Wre - Wim))        # o3
    bw = np.stack(tiles, 1).astype(np.float32)  # [128, 22, 128]
    _cache["bw"] = bw
    return bw


# stage-B matmul list: (bw_idx, operand_name, start, stop) per output stile
# tile order matches _fft8_bweights: [s0, s4, s1t0(4), s1t1(4), s2t0(2),
# s2t1(2), s3t0(4), s3t1(4)] -> indices 0,1, 2-5, 6-9, 10-11, 12-13, 14-21
_BMMS = [
    [(0, "u0", True, True)],
    [(1, "u4", True, True)],
    [(2, "o0", True, False), (3, "o1", False, False),
     (4, "o2", False, False), (5, "o3", False, True)],
    [(6, "o0", True, False), (7, "o1", False, False),
     (8, "o2", False, False), (9, "o3", False, True)],
    [(10, "u2re", True, False), (11, "u2im", False, True)],
    [(12, "u2re", True, False), (13, "u2im", False, True)],
    [(14, "o0", True, False), (15, "o1", False, False),
     (16, "o2", False, False), (17, "o3", False, True)],
    [(18, "o0", True, False), (19, "o1", False, False),
     (20, "o2", False, False), (21, "o3", False, True)],
]
_ST_S = [(0, 0), (4, 0), (1, 0), (1, 1), (2, 0), (2, 1), (3, 0), (3, 1)]


def _fft8_maps():
    """Static per-stile unpack maps: row -> (k, comp, sign)."""
    if "maps" in _cache:
        return _cache["maps"]
    maps = []
    for st in range(NST):
        s, ot = _ST_S[st]
        kk = np.zeros(128, np.int64)
        cc = np.zeros(128, np.int64)
        sg = np.zeros(128, np.float32)
        if s == 0:
            row = 0
            for q in range(65):
                if q in (0, 64):
                    kk[row], cc[row], sg[row] = 8 * q, 0, 1.0
                    row += 1
                else:
                    kk[row], cc[row], sg[row] = 8 * q, 0, 1.0
                    kk[row + 1], cc[row + 1], sg[row + 1] = 8 * q, 1, 1.0
                    row += 2
            assert row == 128
        elif s == 4:
            for qi in range(64):
                kk[2 * qi], cc[2 * qi], sg[2 * qi] = 8 * qi + 4, 0, 1.0
                kk[2 * qi + 1], cc[2 * qi + 1], sg[2 * qi + 1] = 8 * qi + 4, 1, 1.0
        else:
            for qi in range(64):
                q = qi + 64 * ot
                k = 8 * q + s
                if k <= 512:
                    kk[2 * qi], cc[2 * qi], sg[2 * qi] = k, 0, 1.0
                    kk[2 * qi + 1], cc[2 * qi + 1], sg[2 * qi + 1] = k, 1, 1.0
                else:
                    km = 1024 - k
                    assert 0 <= km <= 512
                    kk[2 * qi], cc[2 * qi], sg[2 * qi] = km, 0, 1.0
                    kk[2 * qi + 1], cc[2 * qi + 1], sg[2 * qi + 1] = km, 1, -1.0
        maps.append((kk, cc, sg))
    _cache["maps"] = maps
    return maps


def _fft8_structure(dlnf):
    """Per-core chain structure + L-weight blocks (lhsT [n,tau] layout)."""
    L, idx_lo = _build_L_dense(dlnf)
    cores = []
    for c in range(NCORES):
        chains = []  # (dl, I, [J...], blk_base)
        blocks = []
        for dl in range(D_PER):
            d = D_PER * c + dl
            for I in range(8):
                seg = idx_lo[d, 128 * I : 128 * (I + 1)]
                lo = int(seg.min()) // 128
                hi = int(seg.max() + 1) // 128
                js = list(range(lo, hi + 1))
                chains.append((dl, I, js, len(blocks)))
                for J in js:
                    blk = L[d, 128 * I : 128 * (I + 1), 128 * J : 128 * (J + 1)]
                    blocks.append(np.ascontiguousarray(blk.T))  # [n, tau]
        lw = np.stack(blocks, 1).astype(ml_dtypes.bfloat16)  # [128, nblk, 128]
        cores.append({"chains": chains, "lw": lw, "nblk": len(blocks)})
    return cores


def _build_nc_fft8(struct, iters=1):
    import concourse.bacc as bacc
    import concourse.mybir as mybir
    from concourse import tile

    f32 = mybir.dt.float32
    bf16 = mybir.dt.bfloat16
    nblk = struct["nblk"]
    chains = struct["chains"]

    nc = bacc.Bacc("TRN2", target_bir_lowering=False, debug=False)
    xt_d = nc.dram_tensor("xt", [128, B, 4, MQP], bf16, kind="ExternalInput")
    lw_d = nc.dram_tensor("lw", [128, nblk, 128], bf16, kind="ExternalInput")
    bw_d = nc.dram_tensor("bw", [128, NBW, 128], bf16, kind="ExternalInput")
    # out[slice = 4*dl + b, stile, p, w]
    out_d = nc.dram_tensor("out", [8, NST, 128, NWP], bf16, kind="ExternalOutput")

    A = mybir.AluOpType

    def body(nc, tc, pools):
        (xpool, lwpool, rsb, upool, tpool, xst, rps, rss, bps) = pools
        x_sb = xpool.tile([128, B, 4, MQP], bf16, name="x_sb")
        lw_sb = lwpool.tile([128, nblk, 128], bf16, name="lw_sb")
        bw_sb = lwpool.tile([128, NBW, 128], bf16, name="bw_sb")
        nc.sync.dma_start(lw_sb, lw_d[:])
        for b in range(B):
            nc.sync.dma_start(x_sb[:, b], xt_d[:, b])
        nc.sync.dma_start(bw_sb, bw_d[:])

        rtiles = [None, None]  # per pipeline parity: dict I -> sbuf tile
        utiles = [None, None]

        def emit_R(s):
            dl, b = s // 4, s % 4
            ch = {I: (js, base) for (cdl, I, js, base) in chains if cdl == dl}

            def chain(ps_half, I):
                js, base = ch[I]
                for ci, J in enumerate(js):
                    nc.tensor.matmul(
                        ps_half,
                        lw_sb[:, base + ci],
                        x_sb[:, b, J % 4, J // 4 : J // 4 + NWP],
                        start=(ci == 0),
                        stop=(ci == len(js) - 1),
                    )

            # 2-bank PSUM pairs coarsen the sync grain: one DVE butterfly op
            # and one ACT copy per PAIR instead of per tile.  Longs [r0|r1],
            # [r2|r3] stay in PSUM (the one PSUM operand DVE allows) on a
            # 2-deep rotation; shorts [r4|r5], [r6|r7] are copied to SBUF.
            L01 = rps.tile([128, 2, NWP], f32, name="L01", tag="rp")
            S45p = rss.tile([128, 2, NWP], f32, name="S45p", tag="rq")
            chain(L01[:, 0], 0)
            chain(S45p[:, 0], 4)
            chain(L01[:, 1], 1)
            chain(S45p[:, 1], 5)
            S45 = rsb.tile([128, 2, NWP], bf16, name="s45", tag="s45")
            nc.scalar.copy(S45, S45p)
            L23 = rps.tile([128, 2, NWP], f32, name="L23", tag="rp")
            S67p = rss.tile([128, 2, NWP], f32, name="S67p", tag="rq")
            # L23 chains first: S67p reuses S45p's banks (rss bufs=1), so its
            # first chain must trail the S45->SBUF copy; the two L23 chains
            # in between give that copy exactly the headroom it needs
            chain(L23[:, 0], 2)
            chain(L23[:, 1], 3)
            chain(S67p[:, 0], 6)
            chain(S67p[:, 1], 7)
            S67 = rsb.tile([128, 2, NWP], bf16, name="s67", tag="s67")
            nc.scalar.copy(S67, S67p)
            rtiles[s % 2] = (L01, L23, S45, S67)

        def emit_A(s):
            L01, L23, S45, S67 = rtiles[s % 2]
            tw = {
                nm: tpool.tile([128, 2, NWP], bf16, name=nm, tag=nm)
                for nm in ("e01", "e23", "t12")
            }
            # o-tiles are read by stage B next slice -> double-buffered pool
            o01 = upool.tile([128, 2, NWP], bf16, name="o01", tag="o01")
            o23 = upool.tile([128, 2, NWP], bf16, name="o23", tag="o23")
            u = {
                nm: upool.tile([128, NWP], bf16, name=nm, tag=nm)
                for nm in ("u0", "u4", "u2re", "u2im")
            }
            v = nc.vector
            # level-1 butterflies, double-width: e01=[e0|e1]=[r0+r4|r1+r5]..
            v.tensor_tensor(tw["e01"], L01, S45, op=A.add)
            v.tensor_tensor(o01, L01, S45, op=A.subtract)
            v.tensor_tensor(tw["e23"], L23, S67, op=A.add)
            v.tensor_tensor(o23, L23, S67, op=A.subtract)
            v.tensor_tensor(tw["t12"], tw["e01"], tw["e23"], op=A.add)  # [t1|t2]
            v.tensor_tensor(u["u0"], tw["t12"][:, 0], tw["t12"][:, 1], op=A.add)
            v.tensor_tensor(u["u4"], tw["t12"][:, 0], tw["t12"][:, 1], op=A.subtract)
            v.tensor_tensor(u["u2re"], tw["e01"][:, 0], tw["e23"][:, 0], op=A.subtract)
            v.tensor_tensor(u["u2im"], tw["e23"][:, 1], tw["e01"][:, 1], op=A.subtract)
            # s=1,3 twiddle combines are folded into bw: stage B contracts
            # o0..o3 directly (4 chunks), saving 6 DVE ops per slice
            u["o0"] = o01[:, 0]
            u["o1"] = o01[:, 1]
            u["o2"] = o23[:, 0]
            u["o3"] = o23[:, 1]
            utiles[s % 2] = u

        def emit_B(s):
            u = utiles[s % 2]
            for st in range(NST):
                ps = bps.tile([128, NWP], f32, name="xp", tag="xp")
                for (bwi, unm, sa, so) in _BMMS[st]:
                    nc.tensor.matmul(
                        ps, bw_sb[:, bwi], u[unm], start=sa, stop=so
                    )
                xs = xst.tile([128, NWP], bf16, name="xs", tag="xs")
                # Eviction engine split: DVE evictions queue BEHIND the next
                # slice's stage-A block, so they must be the LAST two stiles
                # (their 2-bank-pool slots aren't needed until the next
                # slice); early stiles evict on ACT, whose queue is timely.
                # (st<3 on DVE stalled B mid-slice; all-ACT overloads ACT.)
                if st >= 6:
                    nc.vector.tensor_copy(xs, ps)
                else:
                    nc.scalar.copy(xs, ps)
                nc.scalar.dma_start(out_d[s, st], xs)

        # Emission order shapes each engine's in-order queue:
        #   PE : R0 R1 [B0 R2] [B1 R3] ... — resample of slice s+2 overlaps
        #        stage-B of slice s, so PE never waits on the DVE.
        #   DVE: A0 A1 ev(B0) A2 ev(B1) ... — A(s+1) is queued BEFORE the
        #        evictions of B(s); evictions never delay the next A block.
        emit_R(0)
        emit_R(1)
        emit_A(0)
        for s in range(8):
            if s + 1 < 8:
                emit_A(s + 1)
            emit_B(s)
            if s + 2 < 8:
                emit_R(s + 2)

    with tile.TileContext(nc) as tc:
        with (
            tc.tile_pool(name="xsb", bufs=2) as xpool,
            tc.tile_pool(name="lwsb", bufs=2) as lwpool,
            tc.tile_pool(name="rsb", bufs=2) as rsb,
            tc.tile_pool(name="usb", bufs=2) as upool,
            tc.tile_pool(name="tsb", bufs=1) as tpool,
            tc.tile_pool(name="xst", bufs=6) as xst,
            tc.tile_pool(name="rps", bufs=2, space="PSUM") as rps,
            tc.tile_pool(name="rss", bufs=1, space="PSUM") as rss,
            tc.tile_pool(name="bps", bufs=2, space="PSUM") as bps,
        ):
            pools = (xpool, lwpool, rsb, upool, tpool, xst, rps, rss, bps)
            if iters > 1:
                with tc.For_i(0, iters, 1):
                    body(nc, tc, pools)
            else:
                body(nc, tc, pools)

    nc.compile()
    return nc


def _get_fft8_ncs(dlnf, iters):
    key = ("fft8", dlnf.tobytes(), iters)
    if key not in _cache:
        if ("fft8s", dlnf.tobytes()) not in _cache:
            _cache[("fft8s", dlnf.tobytes())] = _fft8_structure(dlnf)
        structs = _cache[("fft8s", dlnf.tobytes())]
        _cache[key] = [_build_nc_fft8(s, iters) for s in structs]
    return _cache[key]


def _prep_fft8(x, dlnf):
    x = np.asarray(x, dtype=np.float32)
    dlnf = np.asarray(dlnf, dtype=np.float32)
    skey = ("fft8s", dlnf.tobytes())
    if skey not in _cache:
        _cache[skey] = _fft8_structure(dlnf)
    structs = _cache[skey]
    xt_n = x.reshape(B, MQ, 4, 128).transpose(3, 0, 2, 1)
    xt = np.zeros((128, B, 4, MQP), ml_dtypes.bfloat16)
    xt[:, :, :, :MQ] = xt_n.astype(ml_dtypes.bfloat16)
    xt = np.ascontiguousarray(xt)
    bw = _fft8_bweights().astype(ml_dtypes.bfloat16)
    return [
        {"xt": xt, "lw": np.ascontiguousarray(structs[c]["lw"]), "bw": bw}
        for c in range(NCORES)
    ]


def _assemble_fft8(results):
    maps = _fft8_maps()
    full = np.zeros((B, NW, D, NF, 2), dtype=np.float32)
    for c, r in enumerate(results):
        o = np.asarray(r["out"]).astype(np.float32)  # [8, 8, 128, NWP]
        for dl in range(D_PER):
            d = D_PER * c + dl
            for b in range(B):
                sl = o[4 * dl + b]  # [8, 128, NWP]
                for st in range(NST):
                    kk, cc, sg = maps[st]
                    full[b, :, d, kk, cc] = (
                        sl[st, :, :NW] * sg[:, None]
                    )
    return (
        full.reshape(B, NW, D, NF * 2)
        .view(np.complex64)
        .reshape(B, NW, D, NF)
    )


# --------------------------------------------------------------------------
# device program
# --------------------------------------------------------------------------
def _build_nc(iters=1, sched=None):
    import concourse.bacc as bacc
    import concourse.mybir as mybir
    from concourse import tile

    sched = sched or SCHED
    f32 = mybir.dt.float32
    bf16 = mybir.dt.bfloat16
    mm_dt = mybir.dt.float32r if sched == "base" else bf16

    nc = bacc.Bacc("TRN2", target_bir_lowering=False, debug=False)

    # xt[p, b, r, mq] = x[b, 128*(4*mq + r) + p]  (mq innermost: every
    # matmul moving slice is contiguous)
    xt_d = nc.dram_tensor("xt", [128, B, 4, MQP], mm_dt, kind="ExternalInput")
    # g[p, kc, 1024*d + fe] = G_d[128*kc + p, keep[fe]]
    g_d = nc.dram_tensor("g", [128, KC, D_PER * FE], mm_dt, kind="ExternalInput")
    if sched == "base":
        out_d = nc.dram_tensor(
            "out", [B, NFT // 4, 128, 4, NWP], f32, kind="ExternalOutput"
        )
    else:
        # out[b, fg, p, j, w]: f2e tile ft = 2*fg + j, psum partition p
        out_d = nc.dram_tensor(
            "out", [B, NFT // 2, 128, 2, NWP], bf16, kind="ExternalOutput"
        )

    def body_base(nc, tc, xpool, gpool, spool, ppool):
        x_sb = xpool.tile([128, B, 4, MQP], mm_dt, name="x_sb")
        g_sb = gpool.tile([128, KC, D_PER * FE], mm_dt, name="g_sb")
        for b in range(B):
            nc.sync.dma_start(x_sb[:, b], xt_d[:, b])
        for kc in range(KC):
            nc.sync.dma_start(g_sb[:, kc], g_d[:, kc])

        for b in range(B):
            for ftp in range(NFT // 2):
                st = spool.tile([128, 2, NWP], f32, name="st")
                for jj in range(2):
                    ft = 2 * ftp + jj
                    ps = ppool.tile([128, NWP], f32, name="ps", tag="ps")
                    for h in range(2):
                        for kc in range(KC):
                            q, r = divmod(kc, 4)
                            nc.tensor.matmul(
                                ps[:, h * 256 : (h + 1) * 256],
                                g_sb[:, kc, 128 * ft : 128 * (ft + 1)],
                                x_sb[:, b, r, q + h * 256 : q + h * 256 + 256],
                                start=(kc == 0 and h == 0),
                                stop=(kc == KC - 1 and h == 1),
                            )
                    eng = nc.vector.tensor_copy if ft % 2 == 0 else nc.scalar.copy
                    eng(st[:, jj], ps)
                nc.scalar.dma_start(
                    out_d[b, ftp // 2, :, 2 * (ftp % 2) : 2 * (ftp % 2) + 2], st
                )

    def body_ws(nc, tc, xpool, gpool, spool, ppool):
        x_sb = xpool.tile([128, B, 4, MQP], mm_dt, name="x_sb")
        g_sb = gpool.tile([128, KC, D_PER * FE], mm_dt, name="g_sb")
        nc.sync.dma_start(g_sb[:, 0], g_d[:, 0])
        for b in range(B):
            nc.sync.dma_start(x_sb[:, b], xt_d[:, b])
        for kc in range(1, KC):
            nc.sync.dma_start(g_sb[:, kc], g_d[:, kc])

        st = {}
        for ftp in range(NFT):
            ps = [
                ppool.tile([128, NWP], f32, name=f"ps{b}", tag=f"ps{b}")
                for b in range(B)
            ]
            for kc in range(KC):
                q, r = divmod(kc, 4)
                w_ap = g_sb[:, kc, 128 * ftp : 128 * (ftp + 1)]
                for b in range(B):
                    nc.tensor.matmul(
                        ps[b],
                        w_ap,
                        x_sb[:, b, r, q : q + NWP],
                        start=(kc == 0),
                        stop=(kc == KC - 1),
                    )
            jj = ftp % 2
            if jj == 0:
                for b in range(B):
                    st[b] = spool.tile([128, 2, NWP], bf16, name=f"st{b}")
            for b in range(B):
                eng = nc.vector.tensor_copy if b % 2 == 0 else nc.scalar.copy
                eng(st[b][:, jj], ps[b])
            if jj == 1:
                for b in range(B):
                    nc.scalar.dma_start(out_d[b, ftp // 2], st[b])

    body = body_base if sched == "base" else body_ws

    with tile.TileContext(nc) as tc:
        with (
            tc.tile_pool(name="xsb", bufs=2) as xpool,
            tc.tile_pool(name="gsb", bufs=2) as gpool,
            tc.tile_pool(name="stage", bufs=3) as spool,
            tc.tile_pool(
                name="psum", bufs=8 if sched == "base" else 2, space="PSUM"
            ) as ppool,
        ):
            if iters > 1:
                with tc.For_i(0, iters, 1):
                    body(nc, tc, xpool, gpool, spool, ppool)
            else:
                body(nc, tc, xpool, gpool, spool, ppool)

    nc.compile()
    return nc


def _get_nc(iters=1, sched=None):
    sched = sched or SCHED
    key = ("nc", iters, sched)
    if key not in _cache:
        _cache[key] = _build_nc(iters, sched)
    return _cache[key]


# --------------------------------------------------------------------------
# host prep / assembly
# --------------------------------------------------------------------------
def _prep_arrays(x, dlnf, sched=None):
    """Host prep: G matrices + transposed/sharded device input arrays."""
    sched = sched or SCHED
    dt = np.float32 if sched == "base" else ml_dtypes.bfloat16
    x = np.asarray(x, dtype=np.float32)
    dlnf = np.asarray(dlnf, dtype=np.float32)
    G = _build_G(dlnf)                                     # (16, 1024, 1026)
    xt_n = x.reshape(B, MQ, 4, 128).transpose(3, 0, 2, 1)  # (128, B, 4, MQ)
    xt = np.zeros((128, B, 4, MQP), dt)
    xt[:, :, :, :MQ] = xt_n.astype(dt)
    xt = np.ascontiguousarray(xt)
    Ge = G[:, :, _KEEP]                                    # (16, 1024, 1024)
    g_all = Ge.reshape(D, KC, 128, FE).transpose(2, 1, 0, 3)  # (128,KC,D,FE)
    in_maps = [
        {
            "xt": xt,
            "g": np.ascontiguousarray(
                g_all[:, :, c * D_PER : (c + 1) * D_PER]
                .reshape(128, KC, D_PER * FE)
                .astype(dt)
            ),
        }
        for c in range(NCORES)
    ]
    return in_maps


def _assemble(results, sched=None):
    """per-core out2 -> (B, NW, D, NF) complex64."""
    sched = sched or SCHED
    full = np.zeros((B, NW, D, FW), dtype=np.float32)
    for c, r in enumerate(results):
        o = np.asarray(r["out"]).astype(np.float32)[..., :NW]
        o = o.transpose(0, 4, 1, 3, 2).reshape(B, NW, D_PER, FE)
        for dd in range(D_PER):
            full[:, :, c * D_PER + dd, _KEEP] = o[:, :, dd]
    return full.view(np.complex64).reshape(B, NW, D, NF)


# --------------------------------------------------------------------------
# runner (jitted multi-core executable, cached across kernel() calls)
# --------------------------------------------------------------------------
def _make_sharded(nc, devices=None):
    import jax
    from jax.experimental.shard_map import shard_map
    from jax.sharding import Mesh, PartitionSpec

    from concourse import bass2jax as b2j
    import concourse.mybir as mybir

    b2j.install_neuronx_cc_hook()
    partition_name = nc.partition_id_tensor.name if nc.partition_id_tensor else None

    in_names, out_names, out_avals, zero_outs = [], [], [], []
    for alloc in nc.m.functions[0].allocations:
        if not isinstance(alloc, mybir.MemoryLocationSet):
            continue
        name = alloc.memorylocations[0].name
        if alloc.kind == "ExternalInput":
            if name != partition_name:
                in_names.append(name)
        elif alloc.kind == "ExternalOutput":
            out_names.append(name)
            shape = tuple(alloc.tensor_shape)
            dtype = mybir.dt.np(alloc.dtype)
            out_avals.append(jax.core.ShapedArray(shape, dtype))
            zero_outs.append(np.zeros(shape, dtype))
    all_names = in_names + out_names
    if partition_name is not None:
        all_names = all_names + [partition_name]

    def _body(*args):
        operands = list(args)
        if partition_name is not None:
            operands.append(b2j.partition_id_tensor())
        outs = b2j._bass_exec_p.bind(
            *operands,
            out_avals=tuple(out_avals),
            in_names=tuple(all_names),
            out_names=tuple(out_names),
            lowering_input_output_aliases=(),
            sim_require_finite=True,
            sim_require_nnan=True,
            nc=nc,
        )
        return tuple(outs)

    if devices is None:
        devices = jax.devices()[:NCORES]
    mesh = Mesh(np.asarray(devices), ("core",))
    nin = len(in_names) + len(zero_outs)
    sharded = jax.jit(
        shard_map(
            _body,
            mesh=mesh,
            in_specs=(PartitionSpec("core"),) * nin,
            out_specs=(PartitionSpec("core"),) * len(out_names),
            check_rep=False,
        ),
        keep_unused=True,
    )
    return sharded, in_names, out_names, out_avals, zero_outs


def _get_runner(iters, sched=None):
    sched = sched or SCHED
    key = ("runner", iters, sched)
    if key in _cache:
        return _cache[key]

    import jax

    nc = _get_nc(iters, sched)
    sharded, in_names, out_names, out_avals, zero_outs = _make_sharded(nc)

    def call(in_maps):
        concat_in = [
            np.concatenate([in_maps[c][name] for c in range(NCORES)], axis=0)
            for name in in_names
        ] + [
            np.zeros((NCORES * z.shape[0], *z.shape[1:]), z.dtype)
            for z in zero_outs
        ]
        out_arrs = sharded(*concat_in)
        jax.block_until_ready(out_arrs)
        return [
            {
                name: np.asarray(out_arrs[i]).reshape(
                    NCORES, *out_avals[i].shape
                )[c]
                for i, name in enumerate(out_names)
            }
            for c in range(NCORES)
        ]

    _cache[key] = call
    return call


def _get_fft8_runner(dlnf, iters):
    """Heterogeneous per-core programs: 8 single-device executables."""
    key = ("fft8run", dlnf.tobytes(), iters)
    if key in _cache:
        return _cache[key]

    import jax

    ncs = _get_fft8_ncs(dlnf, iters)
    devices = jax.devices()[:NCORES]
    cores = []
    for c in range(NCORES):
        sharded, in_names, out_names, out_avals, zero_outs = _make_sharded(
            ncs[c], devices=[devices[c]]
        )
        cores.append((sharded, in_names, out_names, out_avals, zero_outs))

    def call(in_maps):
        outs = []
        for c in range(NCORES):
            sharded, in_names, out_names, out_avals, zero_outs = cores[c]
            args = [
                jax.device_put(in_maps[c][n], devices[c]) for n in in_names
            ] + [
                jax.device_put(np.zeros(z.shape, z.dtype), devices[c])
                for z in zero_outs
            ]
            outs.append(sharded(*args))
        jax.block_until_ready(outs)
        results = []
        for c in range(NCORES):
            _, _, out_names, out_avals, _ = cores[c]
            results.append(
                {
                    n: np.asarray(outs[c][i]).reshape(out_avals[i].shape)
                    for i, n in enumerate(out_names)
                }
            )
        return results

    _cache[key] = call
    return call


def kernel(x, dlnf, n_hann_splits=1, **_unused):
    iters = int(os.environ.get("KERNEL_ITERS", "1"))
    dlnf32 = np.asarray(dlnf, dtype=np.float32)
    if SCHED == "fft8":
        try:
            in_maps = _prep_fft8(x, dlnf32)
            call = _get_fft8_runner(dlnf32, iters)
            return _assemble_fft8(call(in_maps))
        except Exception:
            import traceback

            traceback.print_exc()
            # fall through to the dense-G path
    in_maps = _prep_arrays(x, dlnf, sched="bf16ws")
    try:
        call = _get_runner(iters, sched="bf16ws")
        results = call(in_maps)
    except Exception:
        # robust fallback: the reference implementation of the SPMD runner
        from concourse.bass_utils import run_bass_kernel_spmd

        nc = _get_nc(iters, sched="bf16ws")
        res = run_bass_kernel_spmd(nc, in_maps, core_ids=list(range(NCORES)))
        results = res.results
    return _assemble(results, sched="bf16ws")


# --------------------------------------------------------------------------
# benchmarking: jit once, time repeated executions (no retrace/relower)
# --------------------------------------------------------------------------
def _prepare_bench_fft8(x, dlnf, iters):
    import time

    import jax

    dlnf32 = np.asarray(dlnf, dtype=np.float32)
    in_maps = _prep_fft8(x, dlnf32)
    ncs = _get_fft8_ncs(dlnf32, iters)
    devices = jax.devices()[:NCORES]
    cores = []
    for c in range(NCORES):
        sharded, in_names, out_names, out_avals, zero_outs = _make_sharded(
            ncs[c], devices=[devices[c]]
        )
        args = [
            jax.device_put(in_maps[c][n], devices[c]) for n in in_names
        ] + [
            jax.device_put(np.zeros(z.shape, z.dtype), devices[c])
            for z in zero_outs
        ]
        cores.append((sharded, args))
    outs = [s(*a) for s, a in cores]
    jax.block_until_ready(outs)

    def run():
        t0 = time.perf_counter()
        o = [s(*a) for s, a in cores]
        jax.block_until_ready(o)
        return time.perf_counter() - t0

    return run


def prepare_bench(x, dlnf, iters, sched=None):
    """Returns run() -> wall seconds for one execution of the iters-body NEFF."""
    import time

    import jax

    sched = sched or SCHED
    if sched == "fft8":
        try:
            return _prepare_bench_fft8(x, dlnf, iters)
        except Exception:
            import traceback

            traceback.print_exc()
            sched = "bf16ws"  # degrade to the dense path so timing still runs
    in_maps = _prep_arrays(x, dlnf, sched)
    nc = _get_nc(iters, sched)
    sharded, in_names, out_names, out_avals, zero_outs = _make_sharded(nc)
    concat_in = [
        np.concatenate([in_maps[c][name] for c in range(NCORES)], axis=0)
        for name in in_names
    ] + [np.zeros((NCORES * z.shape[0], *z.shape[1:]), z.dtype) for z in zero_outs]
    concat_in = [jax.device_put(a) for a in concat_in]

    out = sharded(*concat_in)
    jax.block_until_ready(out)

    def run():
        t0 = time.perf_counter()
        o = sharded(*concat_in)
        jax.block_until_ready(o)
        return time.perf_counter() - t0

    return run


if __name__ == "__main__":
    rng = np.random.default_rng(0)
    x = rng.standard_normal((B, N), dtype=np.float32)
    dlnf = rng.uniform(-0.5, 0.5, size=(D,)).astype(np.float32)
    out = kernel(x, dlnf, 1)
    print("out:", out.shape, out.dtype)
